# revision 24
# baseline (speedup 1.0000x reference)
"""Trainium2 Bass kernel for nn_EndpointRegressor (2x TransformerConv GNN +
AttentionalAggregation) distributed over 8 NeuronCores.  v2: bf16 datapath.

Sharding: edges partitioned by destination node range (6272 nodes/core);
each core owns its dst nodes exclusively, so segment softmax/scatter stats
need no cross-core reduction.  Per layer each core computes its nodes'
k|v table (384-col bf16 rows, biases + edge bias folded), AllGathers it,
and dma_gathers rows for its edge shard.  The per-edge projection
e = edge_attr @ We is never materialized: its alpha contribution comes via
a node-level C table (C[n,h,c] = q[n,h]·We[c,h], gathered to edges through
the one-hot st_T matmul together with q), and its value contribution via
scattered stats S[n,h,c] = sum_e ex*ea_c followed by a per-window rank-16
correction matmul S @ WeP.  Segment softmax uses exp without max
subtraction (alpha ~ ±0.1 for this model family); the denominator is the
c=4 (ones) column of S.  One-hot scatter/gather matrices are host-built
and streamed as bf16; all matmuls are bf16 (FWL fast-weight-load active),
accumulation stays in fp32 PSUM.
"""
import math
import numpy as np
import ml_dtypes

import concourse.bass as bass
import concourse.bacc as bacc
import concourse.mybir as mybir
import concourse.tile as tile
from concourse._compat import get_trn_type
from concourse.bass_utils import run_bass_kernel_spmd
from concourse.library_config import mlp

# ---- problem constants ----
N, E, G = 50000, 500000, 32
H, D = 4, 40
HID = H * D            # 160
NCORES = 8
NSHARD = 6272          # 49*128 nodes per core
NPAD = NCORES * NSHARD # 50176
WIN = NSHARD // 128    # 49
SPLIT = NPAD // 2      # 25088 (int16 gather indices => 2 tables)
NG = WIN               # one gather per window per stream (num_idxs <= 1024!)
INVSQD = 1.0 / math.sqrt(float(D))

KVROW = 384            # [k 160 | v 160 | pad 64]
QCROW = 192            # [q 160 | C 20 | pad 12]
QCW = QCROW + 161      # + [r 160 | -rQ 1] = 353

f32 = mybir.dt.float32
bf16 = mybir.dt.bfloat16
i16 = mybir.dt.int16
npbf = ml_dtypes.bfloat16

AF = mybir.ActivationFunctionType


def _wrap16(ix):
    """[n] int16 -> [128, n//16] dma_gather index layout (16-wrap, x8 replicate)."""
    return np.tile(ix.reshape(-1, 16).T, (8, 1))


def _preprocess(x, edge_index, edge_attr, batch):
    src = np.asarray(edge_index[0], dtype=np.int64)
    dst = np.asarray(edge_index[1], dtype=np.int64)
    ea = np.asarray(edge_attr, dtype=np.float32)
    order = np.argsort(dst, kind="stable")
    src, dst, ea = src[order], dst[order], ea[order]

    core = dst // NSHARD
    win = (dst % NSHARD) // 128
    low = src < SPLIT

    buckets = {}
    for r in range(NCORES):
        m_r = core == r
        for w in range(WIN):
            m = m_r & (win == w)
            idx = np.nonzero(m)[0]
            buckets[(r, w)] = (idx[low[idx]], idx[~low[idx]])

    C_L = max(1, max((len(b[0]) + 127) // 128 for b in buckets.values()))
    C_H = max(1, max((len(b[1]) + 127) // 128 for b in buckets.values()))
    NCH = C_L + C_H
    GS_L, GS_H = C_L * 128, C_H * 128

    per_core = []
    for r in range(NCORES):
        Lslots = np.zeros(NG * GS_L, np.int64)
        Hslots = np.zeros(NG * GS_H, np.int64)
        eaC = np.zeros((WIN, 128, NCH, 36), np.float32)
        stq = np.zeros((WIN, 128, NCH * 128), np.float32)  # st_T [node, (chunk, edge)]
        sts = np.zeros((WIN, 128, NCH * 128), np.float32)  # st [edge, (chunk, node)]
        for w in range(WIN):
            lo, hi = buckets[(r, w)]
            for (idx_e, slots, Cg, j0, table_off) in (
                (lo, Lslots, C_L, 0, 0),
                (hi, Hslots, C_H, C_L, SPLIT),
            ):
                n = len(idx_e)
                s0 = w * Cg * 128
                slots[s0:s0 + n] = src[idx_e] - table_off
                kk = np.arange(n)
                jj = j0 + kk // 128
                pp = kk % 128
                dr = (dst[idx_e] % 128).astype(np.int64)
                # cols 0:16 (h,c)-major; 16:32 (c,h)-major; 32:36 ones (c=4)
                for h in range(4):
                    eaC[w, pp, jj, h * 4:h * 4 + 4] = ea[idx_e]
                    eaC[w, pp, jj, 32 + h] = 1.0
                eaC[w, pp, jj, 16:32] = np.repeat(ea[idx_e], 4, axis=-1).reshape(-1, 16)
                stq[w, dr, jj * 128 + pp] = 1.0
                sts[w, pp, jj * 128 + dr] = 1.0
        # own-node arrays
        n0 = r * NSHARD
        x6T = np.zeros((6, NSHARD), np.float32)
        x6T[5, :] = 1.0
        sgw = np.zeros((WIN, 128, 32), np.float32)
        n_real = max(0, min(NSHARD, N - n0))
        if n_real > 0:
            x6T[:5, :n_real] = np.asarray(x[n0:n0 + n_real], np.float32).T
            bc = np.asarray(batch[n0:n0 + n_real], np.int64)
            sgw.reshape(NSHARD, 32)[np.arange(n_real), bc] = 1.0
        ws = np.concatenate(
            [stq, sts, eaC.reshape(WIN, 128, NCH * 36)], axis=-1)
        per_core.append(
            dict(
                x6T=x6T.astype(npbf),
                idxL=np.ascontiguousarray(_wrap16(Lslots.astype(np.int16))),
                idxH=np.ascontiguousarray(_wrap16(Hslots.astype(np.int16))),
                ws=np.ascontiguousarray(ws).astype(npbf),
                sgw=sgw.astype(npbf),
            )
        )
    return per_core, C_L, C_H


def _weights(inp):
    """Host-side weight packing (f64 folds -> bf16)."""
    w = {}
    W_in = np.asarray(inp["W_in"], np.float64)
    b_in = np.asarray(inp["b_in"], np.float64)
    for l in range(2):
        Wq, bq = inp["Wq"][l].astype(np.float64), inp["bq"][l].astype(np.float64)
        Wk, bk = inp["Wk"][l].astype(np.float64), inp["bk"][l].astype(np.float64)
        Wv, bv = inp["Wv"][l].astype(np.float64), inp["bv"][l].astype(np.float64)
        We, be = inp["We"][l].astype(np.float64), inp["be"][l].astype(np.float64)
        Wskip, bskip = inp["Wskip"][l].astype(np.float64), inp["bskip"][l].astype(np.float64)
        Wbeta = inp["Wbeta"][l].astype(np.float64)
        P = Wbeta[:HID, 0] + Wbeta[2 * HID:, 0]
        Q = Wbeta[HID:2 * HID, 0] - Wbeta[2 * HID:, 0]
        # WeP [20, 160]: rows (c, h) c<4 -> We[c, h-block]; c=4 rows zero
        WeP = np.zeros((20, HID), np.float64)
        WeC = np.zeros((HID, 16), np.float64)
        for h in range(H):
            for c in range(4):
                WeP[c * 4 + h, h * D:(h + 1) * D] = We[c, h * D:(h + 1) * D]
                WeC[h * D:(h + 1) * D, h * 4 + c] = We[c, h * D:(h + 1) * D]
        if l == 0:
            Wq_e = W_in @ Wq; bq_e = b_in @ Wq + bq
            Wk_e = W_in @ Wk; bk_e = b_in @ Wk + bk + be
            Wv_e = W_in @ Wv; bv_e = b_in @ Wv + bv + be
            Ws_e = W_in @ Wskip; bs_e = b_in @ Wskip + bskip
        else:
            Wq_e, bq_e = Wq, bq
            Wk_e, bk_e = Wk, bk + be
            Wv_e, bv_e = Wv, bv + be
            Ws_e, bs_e = Wskip, bskip
        nin = Wq_e.shape[0]
        kv_slab = np.zeros((nin + 1, KVROW), np.float64)
        kv_slab[:nin, 0:160] = Wk_e
        kv_slab[nin, 0:160] = bk_e
        kv_slab[:nin, 160:320] = Wv_e
        kv_slab[nin, 160:320] = bv_e
        qc_slab = np.zeros((nin + 1, QCW), np.float64)
        qc_slab[:nin, 0:160] = Wq_e
        qc_slab[nin, 0:160] = bq_e
        qc_slab[:nin, 160:176] = Wq_e @ WeC
        qc_slab[nin, 160:176] = bq_e @ WeC
        qc_slab[:nin, QCROW:QCROW + 160] = Ws_e
        qc_slab[nin, QCROW:QCROW + 160] = bs_e
        qc_slab[:nin, QCROW + 160] = -(Ws_e @ Q)
        qc_slab[nin, QCROW + 160] = -(bs_e @ Q)
        if l == 0:
            w["kvslab0"] = kv_slab.astype(npbf)       # [6, 384]
            w["qcslab0"] = qc_slab.astype(npbf)       # [6, 353]
        else:
            w["kvslab1"] = kv_slab.astype(npbf)       # [161, 384]
            w["qcslab1"] = qc_slab.astype(npbf)       # [161, 353]
        w[f"wep{l}"] = WeP.astype(npbf)               # [16, 160]
        w[f"prep{l}"] = np.broadcast_to(P, (128, HID)).astype(npbf).copy()
    w["ident"] = np.eye(128).astype(npbf)
    Wg1 = np.asarray(inp["Wg1"], np.float64)
    w["wg1h1"] = np.concatenate([Wg1[:HID], np.asarray(inp["bg1"], np.float64)[None, :]], 0).astype(npbf)  # [161,160]
    w["wg1h2"] = np.concatenate([Wg1[HID:], np.zeros((1, HID))], 0).astype(npbf)
    w["wg2rep"] = np.broadcast_to(np.asarray(inp["Wg2"], np.float64)[:, 0], (128, HID)).astype(npbf).copy()
    w["bg2rep"] = np.full((128, 1), float(np.asarray(inp["bg2"]).reshape(-1)[0])).astype(npbf)
    w["wh1"] = np.concatenate([np.asarray(inp["Wh1"], np.float64),
                               np.asarray(inp["bh1"], np.float64)[None, :]], 0).astype(npbf)  # [321,320]
    w["wh2"] = np.concatenate([np.asarray(inp["Wh2"], np.float64),
                               np.asarray(inp["bh2"], np.float64)[None, :]], 0).astype(npbf)  # [321,6]
    return w


def _build(C_L, C_H):
    NCH = C_L + C_H
    GS_L, GS_H = C_L * 128, C_H * 128
    assert GS_L <= 1024 and GS_H <= 1024, 'dma_gather num_idxs must be <= 1024'

    nc = bacc.Bacc(get_trn_type() or "TRN2", target_bir_lowering=False)

    d = {}
    d["x6T"] = nc.dram_tensor("x6T", [6, NSHARD], bf16, kind="ExternalInput")
    d["idxL"] = nc.dram_tensor("idxL", [128, NG * GS_L // 16], i16, kind="ExternalInput")
    d["idxH"] = nc.dram_tensor("idxH", [128, NG * GS_H // 16], i16, kind="ExternalInput")
    d["ws"] = nc.dram_tensor("ws", [WIN, 128, NCH * 292], bf16, kind="ExternalInput")
    d["sgw"] = nc.dram_tensor("sgw", [WIN, 128, 32], bf16, kind="ExternalInput")
    wshapes = dict(
        kvslab0=[6, KVROW], qcslab0=[6, QCW],
        kvslab1=[161, KVROW], qcslab1=[161, QCW],
        wep0=[20, HID], wep1=[20, HID], prep0=[128, HID], prep1=[128, HID],
        ident=[128, 128], wg1h1=[161, HID], wg1h2=[161, HID],
        wg2rep=[128, HID], bg2rep=[128, 1], wh1=[321, 320], wh2=[321, 6],
    )
    for k, shp in wshapes.items():
        d[k] = nc.dram_tensor(k, shp, bf16, kind="ExternalInput")
    out_d = nc.dram_tensor("out", [32, 6], f32, kind="ExternalOutput")
    dbg_d = nc.dram_tensor("dbg", [128, KVROW], f32, kind="ExternalOutput")

    kv_own = [nc.dram_tensor(f"kv_own{l}", [NSHARD, KVROW], bf16) for l in range(2)]
    kv_full = [nc.dram_tensor(f"kv_full{l}", [NPAD, KVROW], bf16, addr_space="Shared")
               for l in range(2)]
    hT = [None, nc.dram_tensor("hT1", [HID, NSHARD], bf16),
          nc.dram_tensor("hT2", [HID, NSHARD], bf16)]
    h_nm = [None, nc.dram_tensor("h_nm1", [NSHARD, HID], bf16),
            nc.dram_tensor("h_nm2", [NSHARD, HID], bf16)]
    pool_in = nc.dram_tensor("pool_in", [32, 321], f32)
    pool_out = nc.dram_tensor("pool_out", [32, 321], f32, addr_space="Shared")
    rg = [list(range(NCORES))]

    with tile.TileContext(nc) as tc:
        with (
            tc.tile_pool(name="const", bufs=1) as cst,
            tc.tile_pool(name="sb", bufs=2) as sb,
            tc.tile_pool(name="gath", bufs=3) as gath,
            tc.tile_pool(name="ps", bufs=2, space="PSUM") as ps,
        ):
            nc.gpsimd.load_library(mlp)
            regGS_L = nc.gpsimd.to_reg(GS_L)
            regGS_H = nc.gpsimd.to_reg(GS_H)

            C = {}
            def _load_const(key, part, cols, row0=0, dt=bf16):
                t = cst.tile([part, cols], dt, name=f"c_{key}_{row0}")
                nc.sync.dma_start(out=t[:], in_=d[key][row0:row0 + part, :])
                return t
            C["kvslab0"] = _load_const("kvslab0", 6, KVROW)
            C["qcslab0"] = _load_const("qcslab0", 6, QCW)
            C["kvslab1a"] = _load_const("kvslab1", 128, KVROW)
            C["kvslab1b"] = _load_const("kvslab1", 32, KVROW, 128)
            C["kvslab1c"] = _load_const("kvslab1", 1, KVROW, 160)
            C["qcslab1a"] = _load_const("qcslab1", 128, QCW)
            C["qcslab1b"] = _load_const("qcslab1", 32, QCW, 128)
            C["qcslab1c"] = _load_const("qcslab1", 1, QCW, 160)
            for l in range(2):
                C[f"wep{l}"] = _load_const(f"wep{l}", 20, HID)
                C[f"prep{l}"] = _load_const(f"prep{l}", 128, HID)
            C["ident"] = _load_const("ident", 128, 128)
            for key in ("wg1h1", "wg1h2"):
                C[key + "a"] = _load_const(key, 128, HID)
                C[key + "b"] = _load_const(key, 32, HID, 128)
            C["wg1bias"] = _load_const("wg1h1", 1, HID, 160)
            C["wg2rep"] = _load_const("wg2rep", 128, HID)
            C["bg2rep"] = _load_const("bg2rep", 128, 1)
            C["wh1a"] = _load_const("wh1", 128, 320)
            C["wh1b"] = _load_const("wh1", 128, 320, 128)
            C["wh1c"] = _load_const("wh1", 64, 320, 256)
            C["wh1d"] = _load_const("wh1", 1, 320, 320)
            C["wh2a"] = _load_const("wh2", 128, 6)
            C["wh2b"] = _load_const("wh2", 128, 6, 128)
            C["wh2c"] = _load_const("wh2", 64, 6, 256)
            C["wh2d"] = _load_const("wh2", 1, 6, 320)

            idxLt = cst.tile([128, NG * GS_L // 16], i16, name="idxLt")
            nc.sync.dma_start(out=idxLt[:], in_=d["idxL"][:])
            idxHt = cst.tile([128, NG * GS_H // 16], i16, name="idxHt")
            nc.sync.dma_start(out=idxHt[:], in_=d["idxH"][:])

            ones1 = cst.tile([1, 128], bf16, name="ones1")
            nc.gpsimd.memset(ones1[:], 1.0)

            for layer in range(2):
                # ---- kv GEMM own nodes -> kv_own ----
                with nc.named_scope(f"kv{layer}"):
                    for t in range(WIN):
                        csl = slice(t * 128, (t + 1) * 128)
                        pkv = ps.tile([128, KVROW], f32, tag="kve", bufs=2)
                        if layer == 0:
                            xts = sb.tile([6, 128], bf16, tag="xts", bufs=3)
                            nc.sync.dma_start(out=xts[:], in_=d["x6T"][:, csl])
                            nc.tensor.matmul(pkv[:], xts[:], C["kvslab0"][:],
                                             start=True, stop=True)
                        else:
                            hta = sb.tile([128, 128], bf16, tag="hta", bufs=3)
                            nc.sync.dma_start(out=hta[:], in_=hT[1][0:128, csl])
                            htb = sb.tile([32, 128], bf16, tag="htb", bufs=3)
                            nc.sync.dma_start(out=htb[:], in_=hT[1][128:160, csl])
                            nc.tensor.matmul(pkv[:], hta[:], C["kvslab1a"][:], start=True, stop=False)
                            nc.tensor.matmul(pkv[:], htb[:], C["kvslab1b"][:], start=False, stop=False)
                            nc.tensor.matmul(pkv[:], ones1[:, :128], C["kvslab1c"][:], start=False, stop=True)
                        kvsb = sb.tile([128, KVROW], bf16, tag="kvsb")
                        nc.scalar.activation(out=kvsb[:], in_=pkv[:], func=AF.Copy)
                        nc.sync.dma_start(out=kv_own[layer][csl, :], in_=kvsb[:])
                with nc.named_scope(f"ag{layer}"):
                    nc.gpsimd.collective_compute(
                        "AllGather", mybir.AluOpType.bypass, replica_groups=rg,
                        ins=[kv_own[layer][:]], outs=[kv_full[layer][:]])

                # ---- edge phase ----
                with nc.named_scope(f"edge{layer}"):
                    cur = {"L": -1, "H": -1}
                    cur_tile = {"L": None, "H": None}

                    def _gather(region, gt):
                        if cur[region] == gt:
                            return cur_tile[region]
                        idxt, base, gsz, reg = (
                            (idxLt, 0, GS_L, regGS_L) if region == "L"
                            else (idxHt, SPLIT, GS_H, regGS_H)
                        )
                        gtile = gath.tile([128, gsz // 128, KVROW], bf16, tag="g" + region)
                        nc.gpsimd.dma_gather(
                            gtile[:],
                            kv_full[layer][base:base + SPLIT, :],
                            idxt[:, gt * (gsz // 16):(gt + 1) * (gsz // 16)],
                            num_idxs=gsz, num_idxs_reg=reg, elem_size=KVROW)
                        cur[region] = gt
                        cur_tile[region] = gtile
                        return gtile

                    for w in range(WIN):
                        wsl = slice(w * 128, (w + 1) * 128)
                        # window node GEMM -> q|C|r|-rQ
                        psq = ps.tile([128, QCW], f32, tag="pq", bufs=3)
                        if layer == 0:
                            xts = sb.tile([6, 128], bf16, tag="xts", bufs=3)
                            nc.sync.dma_start(out=xts[:], in_=d["x6T"][:, wsl])
                            nc.tensor.matmul(psq[:], xts[:], C["qcslab0"][:],
                                             start=True, stop=True)
                        else:
                            hta = sb.tile([128, 128], bf16, tag="hta", bufs=3)
                            nc.sync.dma_start(out=hta[:], in_=hT[1][0:128, wsl])
                            htb = sb.tile([32, 128], bf16, tag="htb", bufs=3)
                            nc.sync.dma_start(out=htb[:], in_=hT[1][128:160, wsl])
                            nc.tensor.matmul(psq[:], hta[:], C["qcslab1a"][:], start=True, stop=False)
                            nc.tensor.matmul(psq[:], htb[:], C["qcslab1b"][:], start=False, stop=False)
                            nc.tensor.matmul(psq[:], ones1[:, :128], C["qcslab1c"][:], start=False, stop=True)
                        qc = sb.tile([128, QCW], bf16, tag="qc", bufs=2)
                        nc.scalar.activation(out=qc[:], in_=psq[:], func=AF.Copy)

                        wst = sb.tile([128, NCH * 292], bf16, tag="wst", bufs=3)
                        nc.sync.dma_start(out=wst[:], in_=d["ws"][w])
                        stqt = wst[:, 0:NCH * 128]
                        stst = wst[:, NCH * 128:NCH * 256]
                        eact = wst[:, NCH * 256:NCH * 292].rearrange(
                            "p (j c) -> p j c", c=36)

                        gl = _gather("L", w)
                        gh = _gather("H", w)
                        halfL = 0
                        halfH = 0

                        # per-chunk qC one-hot gather matmuls (2 per PSUM bank)
                        qcg = sb.tile([128, NCH, QCROW], bf16, tag="qcg", bufs=2)
                        for pj in range((NCH + 1) // 2):
                            jn = min(2, NCH - pj * 2)
                            pq = ps.tile([128, 2, QCROW], f32, tag="pq", bufs=3)
                            for s in range(jn):
                                j = pj * 2 + s
                                nc.tensor.matmul(pq[:, s, :],
                                                 stqt[:, j * 128:(j + 1) * 128],
                                                 qc[:, 0:QCROW],
                                                 start=True, stop=True,
                                                 skip_group_check=True)
                            nc.scalar.activation(out=qcg[:, pj * 2:pj * 2 + jn, :],
                                                 in_=pq[:, 0:jn, :], func=AF.Copy)

                        # batched DVE per L/H group -- all plain 3D inner-contiguous
                        stage = sb.tile([128, NCH, 192], bf16, tag="stage", bufs=2)
                        al1 = sb.tile([128, NCH * 4], f32, tag="al1", bufs=2)
                        al2 = sb.tile([128, NCH * 4], f32, tag="al2", bufs=2)
                        al = sb.tile([128, NCH * 4], f32, tag="al", bufs=2)
                        wt = sb.tile([128, NCH, 192], bf16, tag="wt", bufs=2)
                        exg = sb.tile([128, NCH, 192], bf16, tag="exg", bufs=2)
                        for (g0, cnt, gt, half) in ((0, C_L, gl, halfL),
                                                    (C_L, C_H, gh, halfH)):
                            kvg = gt[:, half:half + cnt, :]
                            qs = qcg[:, g0:g0 + cnt, :]
                            # q*k -> stage[.., 0:160]
                            nc.vector.tensor_tensor(
                                out=stage[:, g0:g0 + cnt, 0:160],
                                in0=qs[:, :, 0:160],
                                in1=kvg[:, :, 0:160],
                                op=mybir.AluOpType.mult)
                            # ea*C -> stage[.., 160:176]
                            nc.vector.tensor_tensor(
                                out=stage[:, g0:g0 + cnt, 160:176],
                                in0=qs[:, :, 160:176],
                                in1=eact[:, g0:g0 + cnt, 0:16],
                                op=mybir.AluOpType.mult)
                        # alpha = sum_d q*k + sum_c ea*C (whole window)
                        nc.vector.tensor_reduce(
                            out=al1[:], in_=stage[:, :, 0:160]
                                .rearrange("p j (h dd) -> p j h dd", h=4),
                            axis=mybir.AxisListType.X, op=mybir.AluOpType.add)
                        nc.vector.tensor_reduce(
                            out=al2[:], in_=stage[:, :, 160:176]
                                .rearrange("p j (h c) -> p j h c", h=4),
                            axis=mybir.AxisListType.X, op=mybir.AluOpType.add)
                        nc.vector.tensor_add(al[:], al1[:], al2[:])
                        # exp-expand on ACT (broadcast input APs)
                        nc.scalar.activation(
                            out=exg[:, :, 0:160].rearrange("p j (h dd) -> p j h dd", h=4),
                            in_=al[:].rearrange("p (j h o) -> p j h o", h=4, o=1)
                                 .to_broadcast([128, NCH, 4, 40]),
                            func=AF.Exp, scale=INVSQD)
                        nc.scalar.activation(
                            out=exg[:, :, 160:180].rearrange("p j (c h) -> p j c h", c=5),
                            in_=al[:].rearrange("p (j o h) -> p j o h", o=1, h=4)
                                 .to_broadcast([128, NCH, 5, 4]),
                            func=AF.Exp, scale=INVSQD)
                        for (g0, cnt, gt, half) in ((0, C_L, gl, halfL),
                                                    (C_L, C_H, gh, halfH)):
                            kvg = gt[:, half:half + cnt, :]
                            # wt v-block = v_g * ex
                            nc.vector.tensor_tensor(
                                out=wt[:, g0:g0 + cnt, 0:160],
                                in0=kvg[:, :, 160:320],
                                in1=exg[:, g0:g0 + cnt, 0:160],
                                op=mybir.AluOpType.mult)
                            # wt S-block (c,h)-major incl ones col = ea|1 * ex
                            nc.vector.tensor_tensor(
                                out=wt[:, g0:g0 + cnt, 160:180],
                                in0=eact[:, g0:g0 + cnt, 16:36],
                                in1=exg[:, g0:g0 + cnt, 160:180],
                                op=mybir.AluOpType.mult)

                        # scatter: acc[nodes, (h,48)] += st^T @ wt
                        pacc = ps.tile([128, 192], f32, tag="acc", bufs=2)
                        for j in range(NCH):
                            nc.tensor.matmul(pacc[:],
                                             stst[:, j * 128:(j + 1) * 128],
                                             wt[:, j, :],
                                             start=(j == 0), stop=(j == NCH - 1),
                                             skip_group_check=True)

                        # ---- window post ----
                        accsb = sb.tile([128, 192], bf16, tag="accsb")
                        nc.scalar.activation(out=accsb[:], in_=pacc[:], func=AF.Copy)
                        # S correction: transpose accS [128, (c,h)] -> [20,128]
                        pst = ps.tile([20, 128], bf16, tag="tp", bufs=1)
                        nc.tensor.transpose(pst[:], accsb[:, 160:180], C["ident"][:])
                        tS = sb.tile([20, 128], bf16, tag="tS")
                        nc.scalar.activation(out=tS[:], in_=pst[:], func=AF.Copy)
                        pcorr = ps.tile([128, HID], f32, tag="tp", bufs=1)
                        nc.tensor.matmul(pcorr[:], tS[:], C[f"wep{layer}"][:],
                                         start=True, stop=True)
                        # outn = (accv + corr) * 1/denom
                        outn0 = sb.tile([128, HID], bf16, tag="outn0")
                        nc.vector.tensor_tensor(
                            out=outn0[:], in0=accsb[:, 0:160], in1=pcorr[:],
                            op=mybir.AluOpType.add)
                        dmax = sb.tile([128, 4], f32, tag="dmax")
                        nc.vector.tensor_scalar_max(dmax[:], accsb[:, 176:180], 1e-30)
                        denr = sb.tile([128, 4], f32, tag="denr")
                        nc.vector.reciprocal(out=denr[:], in_=dmax[:])
                        outn = sb.tile([128, HID], bf16, tag="outn")
                        nc.vector.tensor_tensor(
                            out=outn[:].rearrange("p (h dd) -> p h dd", h=4),
                            in0=outn0[:].rearrange("p (h dd) -> p h dd", h=4),
                            in1=denr[:].rearrange("p (h o) -> p h o", o=1)
                                .to_broadcast([128, 4, 40]),
                            op=mybir.AluOpType.mult)
                        # beta gate
                        scr = sb.tile([128, HID], bf16, tag="scr")
                        nc.vector.tensor_tensor(out=scr[:], in0=outn[:],
                                                in1=C[f"prep{layer}"][:],
                                                op=mybir.AluOpType.mult)
                        outP = sb.tile([128, 1], f32, tag="outP")
                        nc.vector.tensor_reduce(
                            out=outP[:], in_=scr[:].rearrange("p (a b) -> p a b", a=1),
                            axis=mybir.AxisListType.XY, op=mybir.AluOpType.add)
                        exb = sb.tile([128, 1], bf16, tag="exb")
                        nc.scalar.activation(out=exb[:], in_=outP[:], func=AF.Exp,
                                             scale=-1.0, bias=qc[:, 352:353])
                        betad = sb.tile([128, 1], bf16, tag="betad")
                        nc.vector.tensor_scalar_add(betad[:], exb[:], 1.0)
                        beta = sb.tile([128, 1], bf16, tag="beta")
                        with nc.allow_low_precision(reason="beta gate bf16 ok"):
                            nc.vector.reciprocal(out=beta[:], in_=betad[:])
                        dvec = sb.tile([128, HID], bf16, tag="dvec")
                        nc.vector.tensor_sub(dvec[:], qc[:, QCROW:QCROW + 160], outn[:])
                        hp = sb.tile([128, HID], bf16, tag="hp")
                        nc.vector.scalar_tensor_tensor(
                            out=hp[:], in0=dvec[:], scalar=beta[:, 0:1], in1=outn[:],
                            op0=mybir.AluOpType.mult, op1=mybir.AluOpType.add)
                        nc.sync.dma_start(out=h_nm[layer + 1][wsl, :], in_=hp[:])
                        ptr1 = ps.tile([128, 128], bf16, tag="tp", bufs=1)
                        nc.tensor.transpose(ptr1[:], hp[:, 0:128], C["ident"][:])
                        t1 = sb.tile([128, 128], bf16, tag="t1")
                        nc.scalar.activation(out=t1[:], in_=ptr1[:], func=AF.Copy)
                        nc.sync.dma_start(out=hT[layer + 1][0:128, wsl], in_=t1[:])
                        ptr2 = ps.tile([32, 128], bf16, tag="tp", bufs=1)
                        nc.tensor.transpose(ptr2[:], hp[:, 128:160], C["ident"][:])
                        t2 = sb.tile([32, 128], bf16, tag="t2")
                        nc.scalar.activation(out=t2[:], in_=ptr2[:], func=AF.Copy)
                        nc.sync.dma_start(out=hT[layer + 1][128:160, wsl], in_=t2[:])

            # ==== final phase: gate + graph pooling + head MLP ====
            with nc.named_scope("final"):
                pgr = ps.tile([32, 321], f32, tag="acc", bufs=2)
                for w in range(WIN):
                    wsl = slice(w * 128, (w + 1) * 128)
                    h1w = sb.tile([128, HID], bf16, tag="h1w")
                    nc.sync.dma_start(out=h1w[:], in_=h_nm[1][wsl, :])
                    h2w = sb.tile([128, HID], bf16, tag="h2w")
                    nc.sync.dma_start(out=h2w[:], in_=h_nm[2][wsl, :])
                    sgt = sb.tile([128, 32], bf16, tag="sgt", bufs=3)
                    nc.sync.dma_start(out=sgt[:], in_=d["sgw"][w])
                    pg = ps.tile([128, HID], f32, tag="kve", bufs=2)
                    first = True
                    for (src_hT, wkey) in ((hT[1], "wg1h1"), (hT[2], "wg1h2")):
                        g_a = sb.tile([128, 128], bf16, tag="hta", bufs=3)
                        nc.sync.dma_start(out=g_a[:], in_=src_hT[0:128, wsl])
                        g_b = sb.tile([32, 128], bf16, tag="htb", bufs=3)
                        nc.sync.dma_start(out=g_b[:], in_=src_hT[128:160, wsl])
                        nc.tensor.matmul(pg[:], g_a[:], C[wkey + "a"][:], start=first, stop=False)
                        first = False
                        nc.tensor.matmul(pg[:], g_b[:], C[wkey + "b"][:], start=False, stop=False)
                    nc.tensor.matmul(pg[:], ones1[:, :128], C["wg1bias"][:], start=False, stop=True)
                    grelu = sb.tile([128, HID], bf16, tag="grelu")
                    nc.scalar.activation(out=grelu[:], in_=pg[:], func=AF.Relu)
                    scr2 = sb.tile([128, HID], bf16, tag="scr")
                    gatec = sb.tile([128, 1], f32, tag="gatec")
                    nc.vector.tensor_tensor(out=scr2[:], in0=grelu[:],
                                            in1=C["wg2rep"][:], op=mybir.AluOpType.mult)
                    nc.vector.tensor_reduce(
                        out=gatec[:], in_=scr2[:].rearrange("p (a b) -> p a b", a=1),
                        axis=mybir.AxisListType.XY, op=mybir.AluOpType.add)
                    ge = sb.tile([128, 1], f32, tag="ge")
                    nc.scalar.activation(out=ge[:], in_=gatec[:], func=AF.Exp,
                                         bias=C["bg2rep"][:, 0:1])
                    wg = sb.tile([128, 321], bf16, tag="wg")
                    nc.vector.tensor_scalar_mul(wg[:, 0:HID], h1w[:], ge[:, 0:1])
                    nc.vector.tensor_scalar_mul(wg[:, HID:2 * HID], h2w[:], ge[:, 0:1])
                    nc.vector.tensor_copy(out=wg[:, 320:321], in_=ge[:])
                    nc.tensor.matmul(pgr[:], sgt[:], wg[:], start=(w == 0),
                                     stop=(w == WIN - 1), skip_group_check=True)
                pg_sb = sb.tile([32, 321], f32, tag="pg_sb")
                nc.vector.tensor_copy(out=pg_sb[:], in_=pgr[:])
                nc.sync.dma_start(out=pool_in[:], in_=pg_sb[:])
                nc.gpsimd.collective_compute(
                    "AllReduce", mybir.AluOpType.add, replica_groups=rg,
                    ins=[pool_in[:]], outs=[pool_out[:]])
                psb = sb.tile([32, 321], f32, tag="psb")
                nc.sync.dma_start(out=psb[:], in_=pool_out[:])
                gden = sb.tile([32, 1], f32, tag="gden")
                nc.vector.tensor_scalar_max(gden[:], psb[:, 320:321], 1e-30)
                gdr = sb.tile([32, 1], f32, tag="gdr")
                nc.vector.reciprocal(out=gdr[:], in_=gden[:])
                pl = sb.tile([32, 320], bf16, tag="pl")
                nc.vector.tensor_scalar_mul(pl[:], psb[:, 0:320], gdr[:, 0:1])

                def _headmm(vin, wa, wb, wc, wd, nout, tagp):
                    pouts = ps.tile([32, nout], f32, tag=tagp, bufs=(3 if tagp == "pq" else 2))
                    for si, (c0, m) in enumerate(((0, 128), (128, 128), (256, 64))):
                        ptt = ps.tile([m, 32], bf16, tag="tp", bufs=1)
                        nc.tensor.transpose(ptt[:], vin[:, c0:c0 + m], C["ident"][0:32, 0:32])
                        tsb = sb.tile([m, 32], bf16, tag="tsb")
                        nc.vector.tensor_copy(out=tsb[:], in_=ptt[:])
                        nc.tensor.matmul(pouts[:], tsb[:], (wa, wb, wc)[si][:m, :],
                                         start=(si == 0), stop=False, skip_group_check=True)
                    nc.tensor.matmul(pouts[:], ones1[:, :32], wd[:],
                                     start=False, stop=True, skip_group_check=True)
                    return pouts

                ph1 = _headmm(pl, C["wh1a"], C["wh1b"], C["wh1c"], C["wh1d"], 320, "pq")
                vrel = sb.tile([32, 320], bf16, tag="vrel")
                nc.scalar.activation(out=vrel[:], in_=ph1[:], func=AF.Relu)
                ph2 = _headmm(vrel, C["wh2a"], C["wh2b"], C["wh2c"], C["wh2d"], 6, "kve")
                osb = sb.tile([32, 6], f32, tag="osb")
                nc.vector.tensor_copy(out=osb[:], in_=ph2[:])
                nc.sync.dma_start(out=out_d[:], in_=osb[:])
                dbgt = sb.tile([128, KVROW], f32, tag="dbgt")
                nc.gpsimd.memset(dbgt[:], 0.0)
                nc.sync.dma_start(out=dbg_d[:], in_=dbgt[:])

    nc.compile()
    return nc


_CACHE = {}
_LAST_RES = None


def kernel(**inputs):
    inputs = {k: np.asarray(v) for k, v in inputs.items()}
    per_core, C_L, C_H = _preprocess(
        inputs["x"], inputs["edge_index"], inputs["edge_attr"], inputs["batch"])
    w = _weights(inputs)
    key = (C_L, C_H)
    if key not in _CACHE:
        _CACHE[key] = _build(C_L, C_H)
    nc = _CACHE[key]
    in_maps = []
    for r in range(NCORES):
        m = dict(w)
        m.update(per_core[r])
        in_maps.append(m)
    import os
    trace = bool(os.environ.get("KERNEL_TRACE"))
    if trace:
        try:
            import axon_prof
            axon_prof.install()
        except Exception:
            trace = False
    res = run_bass_kernel_spmd(nc, in_maps, core_ids=list(range(NCORES)), trace=trace)
    if trace and res.exec_time_ns is not None:
        print(f"HW exec time: {res.exec_time_ns} ns")
        if res.per_core_scope_times:
            for scope, cores in sorted(res.per_core_scope_times.items()):
                print(f"  scope {scope}: {cores}")
    global _LAST_RES
    _LAST_RES = res
    out = res.results[0]["out"]
    return out.reshape(G, 2, 3).astype(np.float32)


# revision 27
# speedup vs baseline: 1.1018x; 1.1018x over previous
"""Trainium2 Bass kernel for nn_EndpointRegressor (2x TransformerConv GNN +
AttentionalAggregation) distributed over 8 NeuronCores.  v2: bf16 datapath.

Sharding: edges partitioned by destination node range (6272 nodes/core);
each core owns its dst nodes exclusively, so segment softmax/scatter stats
need no cross-core reduction.  Per layer each core computes its nodes'
k|v table (384-col bf16 rows, biases + edge bias folded), AllGathers it,
and dma_gathers rows for its edge shard.  The per-edge projection
e = edge_attr @ We is never materialized: its alpha contribution comes via
a node-level C table (C[n,h,c] = q[n,h]·We[c,h], gathered to edges through
the one-hot st_T matmul together with q), and its value contribution via
scattered stats S[n,h,c] = sum_e ex*ea_c followed by a per-window rank-16
correction matmul S @ WeP.  Segment softmax uses exp without max
subtraction (alpha ~ ±0.1 for this model family); the denominator is the
c=4 (ones) column of S.  One-hot scatter/gather matrices are host-built
and streamed as bf16; all matmuls are bf16 (FWL fast-weight-load active),
accumulation stays in fp32 PSUM.
"""
import math
import numpy as np
import ml_dtypes

import concourse.bass as bass
import concourse.bacc as bacc
import concourse.mybir as mybir
import concourse.tile as tile
from concourse._compat import get_trn_type
from concourse.bass_utils import run_bass_kernel_spmd
from concourse.library_config import mlp

# ---- problem constants ----
N, E, G = 50000, 500000, 32
H, D = 4, 40
HID = H * D            # 160
NCORES = 8
NSHARD = 6272          # 49*128 nodes per core
NPAD = NCORES * NSHARD # 50176
WIN = NSHARD // 128    # 49
SPLIT = NPAD // 2      # 25088 (int16 gather indices => 2 tables)
NG = WIN               # one gather per window per stream (num_idxs <= 1024!)
INVSQD = 1.0 / math.sqrt(float(D))

KVROW = 384            # [k 160 | v 160 | pad 64]
QCROW = 192            # [q 160 | C 20 | pad 12]
QCW = QCROW + 161      # + [r 160 | -rQ 1] = 353

f32 = mybir.dt.float32
bf16 = mybir.dt.bfloat16
i16 = mybir.dt.int16
npbf = ml_dtypes.bfloat16

AF = mybir.ActivationFunctionType


def _wrap16(ix):
    """[n] int16 -> [128, n//16] dma_gather index layout (16-wrap, x8 replicate)."""
    return np.tile(ix.reshape(-1, 16).T, (8, 1))


def _preprocess(x, edge_index, edge_attr, batch):
    src = np.asarray(edge_index[0], dtype=np.int64)
    dst = np.asarray(edge_index[1], dtype=np.int64)
    ea = np.asarray(edge_attr, dtype=np.float32)
    order = np.argsort(dst, kind="stable")
    src, dst, ea = src[order], dst[order], ea[order]

    core = dst // NSHARD
    win = (dst % NSHARD) // 128
    low = src < SPLIT

    buckets = {}
    for r in range(NCORES):
        m_r = core == r
        for w in range(WIN):
            m = m_r & (win == w)
            idx = np.nonzero(m)[0]
            buckets[(r, w)] = (idx[low[idx]], idx[~low[idx]])

    C_L = max(1, max((len(b[0]) + 127) // 128 for b in buckets.values()))
    C_H = max(1, max((len(b[1]) + 127) // 128 for b in buckets.values()))
    NCH = C_L + C_H
    GS_L, GS_H = C_L * 128, C_H * 128

    per_core = []
    for r in range(NCORES):
        Lslots = np.zeros(NG * GS_L, np.int64)
        Hslots = np.zeros(NG * GS_H, np.int64)
        eaC = np.zeros((WIN, 128, NCH, 36), np.float32)
        stq = np.zeros((WIN, 128, NCH * 128), np.float32)  # st_T [node, (chunk, edge)]
        sts = np.zeros((WIN, 128, NCH * 128), np.float32)  # st [edge, (chunk, node)]
        for w in range(WIN):
            lo, hi = buckets[(r, w)]
            for (idx_e, slots, Cg, j0, table_off) in (
                (lo, Lslots, C_L, 0, 0),
                (hi, Hslots, C_H, C_L, SPLIT),
            ):
                n = len(idx_e)
                s0 = w * Cg * 128
                slots[s0:s0 + n] = src[idx_e] - table_off
                kk = np.arange(n)
                jj = j0 + kk // 128
                pp = kk % 128
                dr = (dst[idx_e] % 128).astype(np.int64)
                # cols 0:16 (h,c)-major; 16:32 (c,h)-major; 32:36 ones (c=4)
                for h in range(4):
                    eaC[w, pp, jj, h * 4:h * 4 + 4] = ea[idx_e]
                    eaC[w, pp, jj, 32 + h] = 1.0
                eaC[w, pp, jj, 16:32] = np.repeat(ea[idx_e], 4, axis=-1).reshape(-1, 16)
                stq[w, dr, jj * 128 + pp] = 1.0
                sts[w, pp, jj * 128 + dr] = 1.0
        # own-node arrays
        n0 = r * NSHARD
        x6T = np.zeros((6, NSHARD), np.float32)
        x6T[5, :] = 1.0
        sgw = np.zeros((WIN, 128, 32), np.float32)
        n_real = max(0, min(NSHARD, N - n0))
        if n_real > 0:
            x6T[:5, :n_real] = np.asarray(x[n0:n0 + n_real], np.float32).T
            bc = np.asarray(batch[n0:n0 + n_real], np.int64)
            sgw.reshape(NSHARD, 32)[np.arange(n_real), bc] = 1.0
        ws = np.concatenate(
            [stq, sts, eaC.reshape(WIN, 128, NCH * 36)], axis=-1)
        per_core.append(
            dict(
                x6T=x6T.astype(npbf),
                idxL=np.ascontiguousarray(_wrap16(Lslots.astype(np.int16))),
                idxH=np.ascontiguousarray(_wrap16(Hslots.astype(np.int16))),
                ws=np.ascontiguousarray(ws).astype(npbf),
                sgw=sgw.astype(npbf),
            )
        )
    return per_core, C_L, C_H


def _weights(inp):
    """Host-side weight packing (f64 folds -> bf16)."""
    w = {}
    W_in = np.asarray(inp["W_in"], np.float64)
    b_in = np.asarray(inp["b_in"], np.float64)
    for l in range(2):
        Wq, bq = inp["Wq"][l].astype(np.float64), inp["bq"][l].astype(np.float64)
        Wk, bk = inp["Wk"][l].astype(np.float64), inp["bk"][l].astype(np.float64)
        Wv, bv = inp["Wv"][l].astype(np.float64), inp["bv"][l].astype(np.float64)
        We, be = inp["We"][l].astype(np.float64), inp["be"][l].astype(np.float64)
        Wskip, bskip = inp["Wskip"][l].astype(np.float64), inp["bskip"][l].astype(np.float64)
        Wbeta = inp["Wbeta"][l].astype(np.float64)
        P = Wbeta[:HID, 0] + Wbeta[2 * HID:, 0]
        Q = Wbeta[HID:2 * HID, 0] - Wbeta[2 * HID:, 0]
        # WeP [20, 160]: rows (c, h) c<4 -> We[c, h-block]; c=4 rows zero
        WeP = np.zeros((20, HID), np.float64)
        WeC = np.zeros((HID, 16), np.float64)
        for h in range(H):
            for c in range(4):
                WeP[c * 4 + h, h * D:(h + 1) * D] = We[c, h * D:(h + 1) * D]
                WeC[h * D:(h + 1) * D, h * 4 + c] = We[c, h * D:(h + 1) * D]
        if l == 0:
            Wq_e = W_in @ Wq; bq_e = b_in @ Wq + bq
            Wk_e = W_in @ Wk; bk_e = b_in @ Wk + bk + be
            Wv_e = W_in @ Wv; bv_e = b_in @ Wv + bv + be
            Ws_e = W_in @ Wskip; bs_e = b_in @ Wskip + bskip
        else:
            Wq_e, bq_e = Wq, bq
            Wk_e, bk_e = Wk, bk + be
            Wv_e, bv_e = Wv, bv + be
            Ws_e, bs_e = Wskip, bskip
        nin = Wq_e.shape[0]
        kv_slab = np.zeros((nin + 1, KVROW), np.float64)
        kv_slab[:nin, 0:160] = Wk_e
        kv_slab[nin, 0:160] = bk_e
        kv_slab[:nin, 160:320] = Wv_e
        kv_slab[nin, 160:320] = bv_e
        qc_slab = np.zeros((nin + 1, QCW), np.float64)
        qc_slab[:nin, 0:160] = Wq_e
        qc_slab[nin, 0:160] = bq_e
        qc_slab[:nin, 160:176] = Wq_e @ WeC
        qc_slab[nin, 160:176] = bq_e @ WeC
        qc_slab[:nin, QCROW:QCROW + 160] = Ws_e
        qc_slab[nin, QCROW:QCROW + 160] = bs_e
        qc_slab[:nin, QCROW + 160] = -(Ws_e @ Q)
        qc_slab[nin, QCROW + 160] = -(bs_e @ Q)
        if l == 0:
            w["kvslab0"] = kv_slab.astype(npbf)       # [6, 384]
            w["qcslab0"] = qc_slab.astype(npbf)       # [6, 353]
        else:
            w["kvslab1"] = kv_slab.astype(npbf)       # [161, 384]
            w["qcslab1"] = qc_slab.astype(npbf)       # [161, 353]
        w[f"wep{l}"] = WeP.astype(npbf)               # [16, 160]
        w[f"prep{l}"] = np.broadcast_to(P, (128, HID)).astype(npbf).copy()
    w["ident"] = np.eye(128).astype(npbf)
    Wg1 = np.asarray(inp["Wg1"], np.float64)
    w["wg1h1"] = np.concatenate([Wg1[:HID], np.asarray(inp["bg1"], np.float64)[None, :]], 0).astype(npbf)  # [161,160]
    w["wg1h2"] = np.concatenate([Wg1[HID:], np.zeros((1, HID))], 0).astype(npbf)
    w["wg2rep"] = np.broadcast_to(np.asarray(inp["Wg2"], np.float64)[:, 0], (128, HID)).astype(npbf).copy()
    w["bg2rep"] = np.full((128, 1), float(np.asarray(inp["bg2"]).reshape(-1)[0])).astype(npbf)
    w["wh1"] = np.concatenate([np.asarray(inp["Wh1"], np.float64),
                               np.asarray(inp["bh1"], np.float64)[None, :]], 0).astype(npbf)  # [321,320]
    w["wh2"] = np.concatenate([np.asarray(inp["Wh2"], np.float64),
                               np.asarray(inp["bh2"], np.float64)[None, :]], 0).astype(npbf)  # [321,6]
    return w


def _build(C_L, C_H):
    NCH = C_L + C_H
    GS_L, GS_H = C_L * 128, C_H * 128
    assert GS_L <= 1024 and GS_H <= 1024, 'dma_gather num_idxs must be <= 1024'

    nc = bacc.Bacc(get_trn_type() or "TRN2", target_bir_lowering=False)

    d = {}
    d["x6T"] = nc.dram_tensor("x6T", [6, NSHARD], bf16, kind="ExternalInput")
    d["idxL"] = nc.dram_tensor("idxL", [128, NG * GS_L // 16], i16, kind="ExternalInput")
    d["idxH"] = nc.dram_tensor("idxH", [128, NG * GS_H // 16], i16, kind="ExternalInput")
    d["ws"] = nc.dram_tensor("ws", [WIN, 128, NCH * 292], bf16, kind="ExternalInput")
    d["sgw"] = nc.dram_tensor("sgw", [WIN, 128, 32], bf16, kind="ExternalInput")
    wshapes = dict(
        kvslab0=[6, KVROW], qcslab0=[6, QCW],
        kvslab1=[161, KVROW], qcslab1=[161, QCW],
        wep0=[20, HID], wep1=[20, HID], prep0=[128, HID], prep1=[128, HID],
        ident=[128, 128], wg1h1=[161, HID], wg1h2=[161, HID],
        wg2rep=[128, HID], bg2rep=[128, 1], wh1=[321, 320], wh2=[321, 6],
    )
    for k, shp in wshapes.items():
        d[k] = nc.dram_tensor(k, shp, bf16, kind="ExternalInput")
    out_d = nc.dram_tensor("out", [32, 6], f32, kind="ExternalOutput")
    dbg_d = nc.dram_tensor("dbg", [128, KVROW], f32, kind="ExternalOutput")

    kv_own = [nc.dram_tensor(f"kv_own{l}", [NSHARD, KVROW], bf16) for l in range(2)]
    kv_full = [nc.dram_tensor(f"kv_full{l}", [NPAD, KVROW], bf16, addr_space="Shared")
               for l in range(2)]
    hT = [None, nc.dram_tensor("hT1", [HID, NSHARD], bf16),
          nc.dram_tensor("hT2", [HID, NSHARD], bf16)]
    h_nm = [None, nc.dram_tensor("h_nm1", [NSHARD, HID], bf16),
            nc.dram_tensor("h_nm2", [NSHARD, HID], bf16)]
    pool_in = nc.dram_tensor("pool_in", [32, 321], f32)
    pool_out = nc.dram_tensor("pool_out", [32, 321], f32, addr_space="Shared")
    rg = [list(range(NCORES))]

    with tile.TileContext(nc) as tc:
        with (
            tc.tile_pool(name="const", bufs=1) as cst,
            tc.tile_pool(name="sb", bufs=2) as sb,
            tc.tile_pool(name="gath", bufs=4) as gath,
            tc.tile_pool(name="ps", bufs=2, space="PSUM") as ps,
        ):
            nc.gpsimd.load_library(mlp)
            regGS_L = nc.gpsimd.to_reg(GS_L)
            regGS_H = nc.gpsimd.to_reg(GS_H)

            C = {}
            def _load_const(key, part, cols, row0=0, dt=bf16):
                t = cst.tile([part, cols], dt, name=f"c_{key}_{row0}")
                nc.sync.dma_start(out=t[:], in_=d[key][row0:row0 + part, :])
                return t
            C["kvslab0"] = _load_const("kvslab0", 6, KVROW)
            C["qcslab0"] = _load_const("qcslab0", 6, QCW)
            C["kvslab1a"] = _load_const("kvslab1", 128, KVROW)
            C["kvslab1b"] = _load_const("kvslab1", 32, KVROW, 128)
            C["kvslab1c"] = _load_const("kvslab1", 1, KVROW, 160)
            C["qcslab1a"] = _load_const("qcslab1", 128, QCW)
            C["qcslab1b"] = _load_const("qcslab1", 32, QCW, 128)
            C["qcslab1c"] = _load_const("qcslab1", 1, QCW, 160)
            for l in range(2):
                C[f"wep{l}"] = _load_const(f"wep{l}", 20, HID)
                C[f"prep{l}"] = _load_const(f"prep{l}", 128, HID)
            C["ident"] = _load_const("ident", 128, 128)
            for key in ("wg1h1", "wg1h2"):
                C[key + "a"] = _load_const(key, 128, HID)
                C[key + "b"] = _load_const(key, 32, HID, 128)
            C["wg1bias"] = _load_const("wg1h1", 1, HID, 160)
            C["wg2rep"] = _load_const("wg2rep", 128, HID)
            C["bg2rep"] = _load_const("bg2rep", 128, 1)
            C["wh1a"] = _load_const("wh1", 128, 320)
            C["wh1b"] = _load_const("wh1", 128, 320, 128)
            C["wh1c"] = _load_const("wh1", 64, 320, 256)
            C["wh1d"] = _load_const("wh1", 1, 320, 320)
            C["wh2a"] = _load_const("wh2", 128, 6)
            C["wh2b"] = _load_const("wh2", 128, 6, 128)
            C["wh2c"] = _load_const("wh2", 64, 6, 256)
            C["wh2d"] = _load_const("wh2", 1, 6, 320)

            idxLt = cst.tile([128, NG * GS_L // 16], i16, name="idxLt")
            nc.sync.dma_start(out=idxLt[:], in_=d["idxL"][:])
            idxHt = cst.tile([128, NG * GS_H // 16], i16, name="idxHt")
            nc.sync.dma_start(out=idxHt[:], in_=d["idxH"][:])

            ones1 = cst.tile([1, 128], bf16, name="ones1")
            nc.gpsimd.memset(ones1[:], 1.0)
            eps4 = cst.tile([128, 4], f32, name="eps4")
            nc.gpsimd.memset(eps4[:], 1e-30)
            onep = cst.tile([128, 1], bf16, name="onep")
            nc.gpsimd.memset(onep[:], 1.0)
            eps32 = cst.tile([32, 1], f32, name="eps32")
            nc.gpsimd.memset(eps32[:], 1e-30)

            for layer in range(2):
                # ---- kv GEMM own nodes -> kv_own ----
                with nc.named_scope(f"kv{layer}"):
                    for t in range(WIN):
                        csl = slice(t * 128, (t + 1) * 128)
                        pkv = ps.tile([128, KVROW], f32, tag="kve", bufs=2)
                        if layer == 0:
                            xts = sb.tile([6, 128], bf16, tag="xts", bufs=3)
                            nc.sync.dma_start(out=xts[:], in_=d["x6T"][:, csl])
                            nc.tensor.matmul(pkv[:], xts[:], C["kvslab0"][:],
                                             start=True, stop=True)
                        else:
                            hta = sb.tile([128, 128], bf16, tag="hta", bufs=3)
                            nc.sync.dma_start(out=hta[:], in_=hT[1][0:128, csl])
                            htb = sb.tile([32, 128], bf16, tag="htb", bufs=3)
                            nc.sync.dma_start(out=htb[:], in_=hT[1][128:160, csl])
                            nc.tensor.matmul(pkv[:], hta[:], C["kvslab1a"][:], start=True, stop=False)
                            nc.tensor.matmul(pkv[:], htb[:], C["kvslab1b"][:], start=False, stop=False)
                            nc.tensor.matmul(pkv[:], ones1[:, :128], C["kvslab1c"][:], start=False, stop=True)
                        kvsb = sb.tile([128, KVROW], bf16, tag="kvsb")
                        nc.scalar.activation(out=kvsb[:], in_=pkv[:], func=AF.Copy)
                        nc.sync.dma_start(out=kv_own[layer][csl, :], in_=kvsb[:])
                with nc.named_scope(f"ag{layer}"):
                    nc.gpsimd.collective_compute(
                        "AllGather", mybir.AluOpType.bypass, replica_groups=rg,
                        ins=[kv_own[layer][:]], outs=[kv_full[layer][:]])

                # ---- edge phase ----
                with nc.named_scope(f"edge{layer}"):
                    cur = {"L": -1, "H": -1}
                    cur_tile = {"L": None, "H": None}

                    def _gather(region, gt):
                        if cur[region] == gt:
                            return cur_tile[region]
                        idxt, base, gsz, reg = (
                            (idxLt, 0, GS_L, regGS_L) if region == "L"
                            else (idxHt, SPLIT, GS_H, regGS_H)
                        )
                        gtile = gath.tile([128, gsz // 128, KVROW], bf16, tag="g" + region)
                        nc.gpsimd.dma_gather(
                            gtile[:],
                            kv_full[layer][base:base + SPLIT, :],
                            idxt[:, gt * (gsz // 16):(gt + 1) * (gsz // 16)],
                            num_idxs=gsz, num_idxs_reg=reg, elem_size=KVROW)
                        cur[region] = gt
                        cur_tile[region] = gtile
                        return gtile

                    for w in range(WIN):
                        wsl = slice(w * 128, (w + 1) * 128)
                        # window node GEMM -> q|C|r|-rQ
                        psq = ps.tile([128, QCW], f32, tag="pq", bufs=3)
                        if layer == 0:
                            xts = sb.tile([6, 128], bf16, tag="xts", bufs=3)
                            nc.sync.dma_start(out=xts[:], in_=d["x6T"][:, wsl])
                            nc.tensor.matmul(psq[:], xts[:], C["qcslab0"][:],
                                             start=True, stop=True)
                        else:
                            hta = sb.tile([128, 128], bf16, tag="hta", bufs=3)
                            nc.sync.dma_start(out=hta[:], in_=hT[1][0:128, wsl])
                            htb = sb.tile([32, 128], bf16, tag="htb", bufs=3)
                            nc.sync.dma_start(out=htb[:], in_=hT[1][128:160, wsl])
                            nc.tensor.matmul(psq[:], hta[:], C["qcslab1a"][:], start=True, stop=False)
                            nc.tensor.matmul(psq[:], htb[:], C["qcslab1b"][:], start=False, stop=False)
                            nc.tensor.matmul(psq[:], ones1[:, :128], C["qcslab1c"][:], start=False, stop=True)
                        qc = sb.tile([128, QCW], bf16, tag="qc", bufs=2)
                        nc.scalar.activation(out=qc[:], in_=psq[:], func=AF.Copy)

                        wst = sb.tile([128, NCH * 292], bf16, tag="wst", bufs=3)
                        nc.sync.dma_start(out=wst[:], in_=d["ws"][w])
                        stqt = wst[:, 0:NCH * 128]
                        stst = wst[:, NCH * 128:NCH * 256]
                        eact = wst[:, NCH * 256:NCH * 292].rearrange(
                            "p (j c) -> p j c", c=36)

                        gl = _gather("L", w)
                        gh = _gather("H", w)
                        halfL = 0
                        halfH = 0

                        # per-chunk qC one-hot gather matmuls (2 per PSUM bank)
                        qcg = sb.tile([128, NCH, QCROW], bf16, tag="qcg", bufs=2)
                        for pj in range((NCH + 1) // 2):
                            jn = min(2, NCH - pj * 2)
                            pq = ps.tile([128, 2, QCROW], f32, tag="pq", bufs=3)
                            for s in range(jn):
                                j = pj * 2 + s
                                nc.tensor.matmul(pq[:, s, :],
                                                 stqt[:, j * 128:(j + 1) * 128],
                                                 qc[:, 0:QCROW],
                                                 start=True, stop=True,
                                                 skip_group_check=True)
                            nc.scalar.activation(out=qcg[:, pj * 2:pj * 2 + jn, :],
                                                 in_=pq[:, 0:jn, :], func=AF.Copy)

                        # batched DVE per L/H group -- all plain 3D inner-contiguous
                        stage = sb.tile([128, NCH, 192], bf16, tag="stage", bufs=2)
                        al1 = sb.tile([128, NCH * 4], f32, tag="al1", bufs=2)
                        al2 = sb.tile([128, NCH * 4], f32, tag="al2", bufs=2)
                        al = sb.tile([128, NCH * 4], f32, tag="al", bufs=2)
                        wt = sb.tile([128, NCH, 192], bf16, tag="wt", bufs=2)
                        exg = sb.tile([128, NCH, 192], bf16, tag="exg", bufs=2)
                        for (g0, cnt, gt, half) in ((0, C_L, gl, halfL),
                                                    (C_L, C_H, gh, halfH)):
                            kvg = gt[:, half:half + cnt, :]
                            qs = qcg[:, g0:g0 + cnt, :]
                            # q*k -> stage[.., 0:160]
                            nc.vector.tensor_tensor(
                                out=stage[:, g0:g0 + cnt, 0:160],
                                in0=qs[:, :, 0:160],
                                in1=kvg[:, :, 0:160],
                                op=mybir.AluOpType.mult)
                            # ea*C -> stage[.., 160:176]
                            nc.vector.tensor_tensor(
                                out=stage[:, g0:g0 + cnt, 160:176],
                                in0=qs[:, :, 160:176],
                                in1=eact[:, g0:g0 + cnt, 0:16],
                                op=mybir.AluOpType.mult)
                        # alpha = sum_d q*k + sum_c ea*C (whole window)
                        nc.vector.tensor_reduce(
                            out=al1[:], in_=stage[:, :, 0:160]
                                .rearrange("p j (h dd) -> p j h dd", h=4),
                            axis=mybir.AxisListType.X, op=mybir.AluOpType.add)
                        nc.vector.tensor_reduce(
                            out=al2[:], in_=stage[:, :, 160:176]
                                .rearrange("p j (h c) -> p j h c", h=4),
                            axis=mybir.AxisListType.X, op=mybir.AluOpType.add)
                        nc.vector.tensor_add(al[:], al1[:], al2[:])
                        # exp-expand on ACT (broadcast input APs)
                        nc.scalar.activation(
                            out=exg[:, :, 0:160].rearrange("p j (h dd) -> p j h dd", h=4),
                            in_=al[:].rearrange("p (j h o) -> p j h o", h=4, o=1)
                                 .to_broadcast([128, NCH, 4, 40]),
                            func=AF.Exp, scale=INVSQD)
                        nc.scalar.activation(
                            out=exg[:, :, 160:180].rearrange("p j (c h) -> p j c h", c=5),
                            in_=al[:].rearrange("p (j o h) -> p j o h", o=1, h=4)
                                 .to_broadcast([128, NCH, 5, 4]),
                            func=AF.Exp, scale=INVSQD)
                        for (g0, cnt, gt, half) in ((0, C_L, gl, halfL),
                                                    (C_L, C_H, gh, halfH)):
                            kvg = gt[:, half:half + cnt, :]
                            # wt v-block = v_g * ex
                            nc.vector.tensor_tensor(
                                out=wt[:, g0:g0 + cnt, 0:160],
                                in0=kvg[:, :, 160:320],
                                in1=exg[:, g0:g0 + cnt, 0:160],
                                op=mybir.AluOpType.mult)
                            # wt S-block (c,h)-major incl ones col = ea|1 * ex
                            nc.vector.tensor_tensor(
                                out=wt[:, g0:g0 + cnt, 160:180],
                                in0=eact[:, g0:g0 + cnt, 16:36],
                                in1=exg[:, g0:g0 + cnt, 160:180],
                                op=mybir.AluOpType.mult)

                        # scatter: acc[nodes, (h,48)] += st^T @ wt
                        pacc = ps.tile([128, 192], f32, tag="acc", bufs=2)
                        for j in range(NCH):
                            nc.tensor.matmul(pacc[:],
                                             stst[:, j * 128:(j + 1) * 128],
                                             wt[:, j, :],
                                             start=(j == 0), stop=(j == NCH - 1),
                                             skip_group_check=True)

                        # ---- window post ----
                        accsb = sb.tile([128, 192], bf16, tag="accsb")
                        nc.scalar.activation(out=accsb[:], in_=pacc[:], func=AF.Copy)
                        # S correction: transpose accS [128, (c,h)] -> [20,128]
                        pst = ps.tile([20, 128], bf16, tag="tp", bufs=1)
                        nc.tensor.transpose(pst[:], accsb[:, 160:180], C["ident"][:])
                        tS = sb.tile([20, 128], bf16, tag="tS")
                        nc.scalar.activation(out=tS[:], in_=pst[:], func=AF.Copy)
                        pcorr = ps.tile([128, HID], f32, tag="tp", bufs=1)
                        nc.tensor.matmul(pcorr[:], tS[:], C[f"wep{layer}"][:],
                                         start=True, stop=True)
                        # outn = (accv + corr) * 1/denom
                        outn0 = sb.tile([128, HID], bf16, tag="outn0")
                        nc.vector.tensor_tensor(
                            out=outn0[:], in0=accsb[:, 0:160], in1=pcorr[:],
                            op=mybir.AluOpType.add)
                        dmax = sb.tile([128, 4], f32, tag="dmax")
                        nc.vector.tensor_tensor(out=dmax[:], in0=accsb[:, 176:180],
                                                in1=eps4[:], op=mybir.AluOpType.max)
                        denr = sb.tile([128, 4], f32, tag="denr")
                        nc.vector.reciprocal(out=denr[:], in_=dmax[:])
                        outn = sb.tile([128, HID], bf16, tag="outn")
                        nc.vector.tensor_tensor(
                            out=outn[:].rearrange("p (h dd) -> p h dd", h=4),
                            in0=outn0[:].rearrange("p (h dd) -> p h dd", h=4),
                            in1=denr[:].rearrange("p (h o) -> p h o", o=1)
                                .to_broadcast([128, 4, 40]),
                            op=mybir.AluOpType.mult)
                        # beta gate
                        scr = sb.tile([128, HID], bf16, tag="scr")
                        nc.vector.tensor_tensor(out=scr[:], in0=outn[:],
                                                in1=C[f"prep{layer}"][:],
                                                op=mybir.AluOpType.mult)
                        outP = sb.tile([128, 1], f32, tag="outP")
                        nc.vector.tensor_reduce(
                            out=outP[:], in_=scr[:].rearrange("p (a b) -> p a b", a=1),
                            axis=mybir.AxisListType.XY, op=mybir.AluOpType.add)
                        exb = sb.tile([128, 1], bf16, tag="exb")
                        nc.scalar.activation(out=exb[:], in_=outP[:], func=AF.Exp,
                                             scale=-1.0, bias=qc[:, 352:353])
                        betad = sb.tile([128, 1], bf16, tag="betad")
                        nc.vector.tensor_tensor(out=betad[:], in0=exb[:], in1=onep[:],
                                                op=mybir.AluOpType.add)
                        beta = sb.tile([128, 1], bf16, tag="beta")
                        with nc.allow_low_precision(reason="beta gate bf16 ok"):
                            nc.vector.reciprocal(out=beta[:], in_=betad[:])
                        dvec = sb.tile([128, HID], bf16, tag="dvec")
                        nc.vector.tensor_sub(dvec[:], qc[:, QCROW:QCROW + 160], outn[:])
                        hp = sb.tile([128, HID], bf16, tag="hp")
                        nc.vector.scalar_tensor_tensor(
                            out=hp[:], in0=dvec[:], scalar=beta[:, 0:1], in1=outn[:],
                            op0=mybir.AluOpType.mult, op1=mybir.AluOpType.add)
                        nc.sync.dma_start(out=h_nm[layer + 1][wsl, :], in_=hp[:])
                        ptr1 = ps.tile([128, 128], bf16, tag="tp", bufs=1)
                        nc.tensor.transpose(ptr1[:], hp[:, 0:128], C["ident"][:])
                        t1 = sb.tile([128, 128], bf16, tag="t1")
                        nc.scalar.activation(out=t1[:], in_=ptr1[:], func=AF.Copy)
                        nc.sync.dma_start(out=hT[layer + 1][0:128, wsl], in_=t1[:])
                        ptr2 = ps.tile([32, 128], bf16, tag="tp", bufs=1)
                        nc.tensor.transpose(ptr2[:], hp[:, 128:160], C["ident"][:])
                        t2 = sb.tile([32, 128], bf16, tag="t2")
                        nc.scalar.activation(out=t2[:], in_=ptr2[:], func=AF.Copy)
                        nc.sync.dma_start(out=hT[layer + 1][128:160, wsl], in_=t2[:])

            # ==== final phase: gate + graph pooling + head MLP ====
            with nc.named_scope("final"):
                pgr = ps.tile([32, 321], f32, tag="acc", bufs=2)
                for w in range(WIN):
                    wsl = slice(w * 128, (w + 1) * 128)
                    h1w = sb.tile([128, HID], bf16, tag="h1w")
                    nc.sync.dma_start(out=h1w[:], in_=h_nm[1][wsl, :])
                    h2w = sb.tile([128, HID], bf16, tag="h2w")
                    nc.sync.dma_start(out=h2w[:], in_=h_nm[2][wsl, :])
                    sgt = sb.tile([128, 32], bf16, tag="sgt", bufs=3)
                    nc.sync.dma_start(out=sgt[:], in_=d["sgw"][w])
                    pg = ps.tile([128, HID], f32, tag="kve", bufs=2)
                    first = True
                    for (src_hT, wkey) in ((hT[1], "wg1h1"), (hT[2], "wg1h2")):
                        g_a = sb.tile([128, 128], bf16, tag="hta", bufs=3)
                        nc.sync.dma_start(out=g_a[:], in_=src_hT[0:128, wsl])
                        g_b = sb.tile([32, 128], bf16, tag="htb", bufs=3)
                        nc.sync.dma_start(out=g_b[:], in_=src_hT[128:160, wsl])
                        nc.tensor.matmul(pg[:], g_a[:], C[wkey + "a"][:], start=first, stop=False)
                        first = False
                        nc.tensor.matmul(pg[:], g_b[:], C[wkey + "b"][:], start=False, stop=False)
                    nc.tensor.matmul(pg[:], ones1[:, :128], C["wg1bias"][:], start=False, stop=True)
                    grelu = sb.tile([128, HID], bf16, tag="grelu")
                    nc.scalar.activation(out=grelu[:], in_=pg[:], func=AF.Relu)
                    scr2 = sb.tile([128, HID], bf16, tag="scr")
                    gatec = sb.tile([128, 1], f32, tag="gatec")
                    nc.vector.tensor_tensor(out=scr2[:], in0=grelu[:],
                                            in1=C["wg2rep"][:], op=mybir.AluOpType.mult)
                    nc.vector.tensor_reduce(
                        out=gatec[:], in_=scr2[:].rearrange("p (a b) -> p a b", a=1),
                        axis=mybir.AxisListType.XY, op=mybir.AluOpType.add)
                    ge = sb.tile([128, 1], f32, tag="ge")
                    nc.scalar.activation(out=ge[:], in_=gatec[:], func=AF.Exp,
                                         bias=C["bg2rep"][:, 0:1])
                    wg = sb.tile([128, 321], bf16, tag="wg")
                    nc.vector.tensor_tensor(
                        out=wg[:, 0:HID], in0=h1w[:],
                        in1=ge[:].to_broadcast([128, HID]), op=mybir.AluOpType.mult)
                    nc.vector.tensor_tensor(
                        out=wg[:, HID:2 * HID], in0=h2w[:],
                        in1=ge[:].to_broadcast([128, HID]), op=mybir.AluOpType.mult)
                    nc.vector.tensor_copy(out=wg[:, 320:321], in_=ge[:])
                    nc.tensor.matmul(pgr[:], sgt[:], wg[:], start=(w == 0),
                                     stop=(w == WIN - 1), skip_group_check=True)
                pg_sb = sb.tile([32, 321], f32, tag="pg_sb")
                nc.vector.tensor_copy(out=pg_sb[:], in_=pgr[:])
                nc.sync.dma_start(out=pool_in[:], in_=pg_sb[:])
                nc.gpsimd.collective_compute(
                    "AllReduce", mybir.AluOpType.add, replica_groups=rg,
                    ins=[pool_in[:]], outs=[pool_out[:]])
                psb = sb.tile([32, 321], f32, tag="psb")
                nc.sync.dma_start(out=psb[:], in_=pool_out[:])
                gden = sb.tile([32, 1], f32, tag="gden")
                nc.vector.tensor_tensor(out=gden[:], in0=psb[:, 320:321],
                                        in1=eps32[:], op=mybir.AluOpType.max)
                gdr = sb.tile([32, 1], f32, tag="gdr")
                nc.vector.reciprocal(out=gdr[:], in_=gden[:])
                pl = sb.tile([32, 320], bf16, tag="pl")
                nc.vector.tensor_tensor(
                    out=pl[:], in0=psb[:, 0:320],
                    in1=gdr[:].to_broadcast([32, 320]), op=mybir.AluOpType.mult)

                def _headmm(vin, wa, wb, wc, wd, nout, tagp):
                    pouts = ps.tile([32, nout], f32, tag=tagp, bufs=(3 if tagp == "pq" else 2))
                    for si, (c0, m) in enumerate(((0, 128), (128, 128), (256, 64))):
                        ptt = ps.tile([m, 32], bf16, tag="tp", bufs=1)
                        nc.tensor.transpose(ptt[:], vin[:, c0:c0 + m], C["ident"][0:32, 0:32])
                        tsb = sb.tile([m, 32], bf16, tag="tsb")
                        nc.vector.tensor_copy(out=tsb[:], in_=ptt[:])
                        nc.tensor.matmul(pouts[:], tsb[:], (wa, wb, wc)[si][:m, :],
                                         start=(si == 0), stop=False, skip_group_check=True)
                    nc.tensor.matmul(pouts[:], ones1[:, :32], wd[:],
                                     start=False, stop=True, skip_group_check=True)
                    return pouts

                ph1 = _headmm(pl, C["wh1a"], C["wh1b"], C["wh1c"], C["wh1d"], 320, "pq")
                vrel = sb.tile([32, 320], bf16, tag="vrel")
                nc.scalar.activation(out=vrel[:], in_=ph1[:], func=AF.Relu)
                ph2 = _headmm(vrel, C["wh2a"], C["wh2b"], C["wh2c"], C["wh2d"], 6, "kve")
                osb = sb.tile([32, 6], f32, tag="osb")
                nc.vector.tensor_copy(out=osb[:], in_=ph2[:])
                nc.sync.dma_start(out=out_d[:], in_=osb[:])
                dbgt = sb.tile([128, KVROW], f32, tag="dbgt")
                nc.gpsimd.memset(dbgt[:], 0.0)
                nc.sync.dma_start(out=dbg_d[:], in_=dbgt[:])

    nc.compile()
    return nc


_CACHE = {}
_LAST_RES = None


def kernel(**inputs):
    inputs = {k: np.asarray(v) for k, v in inputs.items()}
    per_core, C_L, C_H = _preprocess(
        inputs["x"], inputs["edge_index"], inputs["edge_attr"], inputs["batch"])
    w = _weights(inputs)
    key = (C_L, C_H)
    if key not in _CACHE:
        _CACHE[key] = _build(C_L, C_H)
    nc = _CACHE[key]
    in_maps = []
    for r in range(NCORES):
        m = dict(w)
        m.update(per_core[r])
        in_maps.append(m)
    import os
    trace = bool(os.environ.get("KERNEL_TRACE"))
    if trace:
        try:
            import axon_prof
            axon_prof.install()
        except Exception:
            trace = False
    res = run_bass_kernel_spmd(nc, in_maps, core_ids=list(range(NCORES)), trace=trace)
    if trace and res.exec_time_ns is not None:
        print(f"HW exec time: {res.exec_time_ns} ns")
        if res.per_core_scope_times:
            for scope, cores in sorted(res.per_core_scope_times.items()):
                print(f"  scope {scope}: {cores}")
    global _LAST_RES
    _LAST_RES = res
    out = res.results[0]["out"]
    return out.reshape(G, 2, 3).astype(np.float32)


# revision 28
# speedup vs baseline: 1.1249x; 1.0210x over previous
"""Trainium2 Bass kernel for nn_EndpointRegressor (2x TransformerConv GNN +
AttentionalAggregation) distributed over 8 NeuronCores.  v2: bf16 datapath.

Sharding: edges partitioned by destination node range (6272 nodes/core);
each core owns its dst nodes exclusively, so segment softmax/scatter stats
need no cross-core reduction.  Per layer each core computes its nodes'
k|v table (384-col bf16 rows, biases + edge bias folded), AllGathers it,
and dma_gathers rows for its edge shard.  The per-edge projection
e = edge_attr @ We is never materialized: its alpha contribution comes via
a node-level C table (C[n,h,c] = q[n,h]·We[c,h], gathered to edges through
the one-hot st_T matmul together with q), and its value contribution via
scattered stats S[n,h,c] = sum_e ex*ea_c followed by a per-window rank-16
correction matmul S @ WeP.  Segment softmax uses exp without max
subtraction (alpha ~ ±0.1 for this model family); the denominator is the
c=4 (ones) column of S.  One-hot scatter/gather matrices are host-built
and streamed as bf16; all matmuls are bf16 (FWL fast-weight-load active),
accumulation stays in fp32 PSUM.
"""
import math
import numpy as np
import ml_dtypes

import concourse.bass as bass
import concourse.bacc as bacc
import concourse.mybir as mybir
import concourse.tile as tile
from concourse._compat import get_trn_type
from concourse.bass_utils import run_bass_kernel_spmd
from concourse.library_config import mlp

# ---- problem constants ----
N, E, G = 50000, 500000, 32
H, D = 4, 40
HID = H * D            # 160
NCORES = 8
NSHARD = 6272          # 49*128 nodes per core
NPAD = NCORES * NSHARD # 50176
WIN = NSHARD // 128    # 49
SPLIT = NPAD // 2      # 25088 (int16 gather indices => 2 tables)
NG = WIN               # one gather per window per stream (num_idxs <= 1024!)
INVSQD = 1.0 / math.sqrt(float(D))

KVROW = 384            # [k 160 | v 160 | pad 64]
QCROW = 192            # [q 160 | C 20 | pad 12]
QCW = QCROW + 161      # + [r 160 | -rQ 1] = 353

f32 = mybir.dt.float32
bf16 = mybir.dt.bfloat16
i16 = mybir.dt.int16
npbf = ml_dtypes.bfloat16

AF = mybir.ActivationFunctionType


def _wrap16(ix):
    """[n] int16 -> [128, n//16] dma_gather index layout (16-wrap, x8 replicate)."""
    return np.tile(ix.reshape(-1, 16).T, (8, 1))


def _preprocess(x, edge_index, edge_attr, batch):
    src = np.asarray(edge_index[0], dtype=np.int64)
    dst = np.asarray(edge_index[1], dtype=np.int64)
    ea = np.asarray(edge_attr, dtype=np.float32)
    order = np.argsort(dst, kind="stable")
    src, dst, ea = src[order], dst[order], ea[order]

    core = dst // NSHARD
    win = (dst % NSHARD) // 128
    # 2-slab interleaved kv table: table row for global node g (core r, local p):
    #   p < 3136: row = r*3136 + p          (L table, rows 0:25088)
    #   p >= 3136: row = 25088 + r*3136 + (p-3136)   (H table)
    _p = src % NSHARD
    _r = src // NSHARD
    srow = np.where(_p < 3136, _r * 3136 + _p, SPLIT + _r * 3136 + (_p - 3136))
    low = srow < SPLIT

    buckets = {}
    for r in range(NCORES):
        m_r = core == r
        for w in range(WIN):
            m = m_r & (win == w)
            idx = np.nonzero(m)[0]
            buckets[(r, w)] = (idx[low[idx]], idx[~low[idx]])

    C_L = max(1, max((len(b[0]) + 127) // 128 for b in buckets.values()))
    C_H = max(1, max((len(b[1]) + 127) // 128 for b in buckets.values()))
    NCH = C_L + C_H
    GS_L, GS_H = C_L * 128, C_H * 128

    per_core = []
    for r in range(NCORES):
        Lslots = np.zeros(NG * GS_L, np.int64)
        Hslots = np.zeros(NG * GS_H, np.int64)
        eaC = np.zeros((WIN, 128, NCH, 36), np.float32)
        stq = np.zeros((WIN, 128, NCH * 128), np.float32)  # st_T [node, (chunk, edge)]
        sts = np.zeros((WIN, 128, NCH * 128), np.float32)  # st [edge, (chunk, node)]
        for w in range(WIN):
            lo, hi = buckets[(r, w)]
            for (idx_e, slots, Cg, j0, table_off) in (
                (lo, Lslots, C_L, 0, 0),
                (hi, Hslots, C_H, C_L, SPLIT),
            ):
                n = len(idx_e)
                s0 = w * Cg * 128
                slots[s0:s0 + n] = srow[idx_e] - table_off
                kk = np.arange(n)
                jj = j0 + kk // 128
                pp = kk % 128
                dr = (dst[idx_e] % 128).astype(np.int64)
                # cols 0:16 (h,c)-major; 16:32 (c,h)-major; 32:36 ones (c=4)
                for h in range(4):
                    eaC[w, pp, jj, h * 4:h * 4 + 4] = ea[idx_e]
                    eaC[w, pp, jj, 32 + h] = 1.0
                eaC[w, pp, jj, 16:32] = np.repeat(ea[idx_e], 4, axis=-1).reshape(-1, 16)
                stq[w, dr, jj * 128 + pp] = 1.0
                sts[w, pp, jj * 128 + dr] = 1.0
        # own-node arrays
        n0 = r * NSHARD
        x6T = np.zeros((6, NSHARD), np.float32)
        x6T[5, :] = 1.0
        sgw = np.zeros((WIN, 128, 32), np.float32)
        n_real = max(0, min(NSHARD, N - n0))
        if n_real > 0:
            x6T[:5, :n_real] = np.asarray(x[n0:n0 + n_real], np.float32).T
            bc = np.asarray(batch[n0:n0 + n_real], np.int64)
            sgw.reshape(NSHARD, 32)[np.arange(n_real), bc] = 1.0
        ws = np.concatenate(
            [stq, sts, eaC.reshape(WIN, 128, NCH * 36)], axis=-1)
        per_core.append(
            dict(
                x6T=x6T.astype(npbf),
                idxL=np.ascontiguousarray(_wrap16(Lslots.astype(np.int16))),
                idxH=np.ascontiguousarray(_wrap16(Hslots.astype(np.int16))),
                ws=np.ascontiguousarray(ws).astype(npbf),
                sgw=sgw.astype(npbf),
            )
        )
    return per_core, C_L, C_H


def _weights(inp):
    """Host-side weight packing (f64 folds -> bf16)."""
    w = {}
    W_in = np.asarray(inp["W_in"], np.float64)
    b_in = np.asarray(inp["b_in"], np.float64)
    for l in range(2):
        Wq, bq = inp["Wq"][l].astype(np.float64), inp["bq"][l].astype(np.float64)
        Wk, bk = inp["Wk"][l].astype(np.float64), inp["bk"][l].astype(np.float64)
        Wv, bv = inp["Wv"][l].astype(np.float64), inp["bv"][l].astype(np.float64)
        We, be = inp["We"][l].astype(np.float64), inp["be"][l].astype(np.float64)
        Wskip, bskip = inp["Wskip"][l].astype(np.float64), inp["bskip"][l].astype(np.float64)
        Wbeta = inp["Wbeta"][l].astype(np.float64)
        P = Wbeta[:HID, 0] + Wbeta[2 * HID:, 0]
        Q = Wbeta[HID:2 * HID, 0] - Wbeta[2 * HID:, 0]
        # WeP [20, 160]: rows (c, h) c<4 -> We[c, h-block]; c=4 rows zero
        WeP = np.zeros((20, HID), np.float64)
        WeC = np.zeros((HID, 16), np.float64)
        for h in range(H):
            for c in range(4):
                WeP[c * 4 + h, h * D:(h + 1) * D] = We[c, h * D:(h + 1) * D]
                WeC[h * D:(h + 1) * D, h * 4 + c] = We[c, h * D:(h + 1) * D]
        if l == 0:
            Wq_e = W_in @ Wq; bq_e = b_in @ Wq + bq
            Wk_e = W_in @ Wk; bk_e = b_in @ Wk + bk + be
            Wv_e = W_in @ Wv; bv_e = b_in @ Wv + bv + be
            Ws_e = W_in @ Wskip; bs_e = b_in @ Wskip + bskip
        else:
            Wq_e, bq_e = Wq, bq
            Wk_e, bk_e = Wk, bk + be
            Wv_e, bv_e = Wv, bv + be
            Ws_e, bs_e = Wskip, bskip
        nin = Wq_e.shape[0]
        kv_slab = np.zeros((nin + 1, KVROW), np.float64)
        kv_slab[:nin, 0:160] = Wk_e
        kv_slab[nin, 0:160] = bk_e
        kv_slab[:nin, 160:320] = Wv_e
        kv_slab[nin, 160:320] = bv_e
        qc_slab = np.zeros((nin + 1, QCW), np.float64)
        qc_slab[:nin, 0:160] = Wq_e
        qc_slab[nin, 0:160] = bq_e
        qc_slab[:nin, 160:176] = Wq_e @ WeC
        qc_slab[nin, 160:176] = bq_e @ WeC
        qc_slab[:nin, QCROW:QCROW + 160] = Ws_e
        qc_slab[nin, QCROW:QCROW + 160] = bs_e
        qc_slab[:nin, QCROW + 160] = -(Ws_e @ Q)
        qc_slab[nin, QCROW + 160] = -(bs_e @ Q)
        if l == 0:
            w["kvslab0"] = kv_slab.astype(npbf)       # [6, 384]
            w["qcslab0"] = qc_slab.astype(npbf)       # [6, 353]
        else:
            w["kvslab1"] = kv_slab.astype(npbf)       # [161, 384]
            w["qcslab1"] = qc_slab.astype(npbf)       # [161, 353]
        w[f"wep{l}"] = WeP.astype(npbf)               # [16, 160]
        w[f"prep{l}"] = np.broadcast_to(P, (128, HID)).astype(npbf).copy()
    w["ident"] = np.eye(128).astype(npbf)
    Wg1 = np.asarray(inp["Wg1"], np.float64)
    w["wg1h1"] = np.concatenate([Wg1[:HID], np.asarray(inp["bg1"], np.float64)[None, :]], 0).astype(npbf)  # [161,160]
    w["wg1h2"] = np.concatenate([Wg1[HID:], np.zeros((1, HID))], 0).astype(npbf)
    w["wg2rep"] = np.broadcast_to(np.asarray(inp["Wg2"], np.float64)[:, 0], (128, HID)).astype(npbf).copy()
    w["bg2rep"] = np.full((128, 1), float(np.asarray(inp["bg2"]).reshape(-1)[0])).astype(npbf)
    w["wh1"] = np.concatenate([np.asarray(inp["Wh1"], np.float64),
                               np.asarray(inp["bh1"], np.float64)[None, :]], 0).astype(npbf)  # [321,320]
    w["wh2"] = np.concatenate([np.asarray(inp["Wh2"], np.float64),
                               np.asarray(inp["bh2"], np.float64)[None, :]], 0).astype(npbf)  # [321,6]
    return w


def _build(C_L, C_H):
    NCH = C_L + C_H
    GS_L, GS_H = C_L * 128, C_H * 128
    assert GS_L <= 1024 and GS_H <= 1024, 'dma_gather num_idxs must be <= 1024'

    nc = bacc.Bacc(get_trn_type() or "TRN2", target_bir_lowering=False)

    d = {}
    d["x6T"] = nc.dram_tensor("x6T", [6, NSHARD], bf16, kind="ExternalInput")
    d["idxL"] = nc.dram_tensor("idxL", [128, NG * GS_L // 16], i16, kind="ExternalInput")
    d["idxH"] = nc.dram_tensor("idxH", [128, NG * GS_H // 16], i16, kind="ExternalInput")
    d["ws"] = nc.dram_tensor("ws", [WIN, 128, NCH * 292], bf16, kind="ExternalInput")
    d["sgw"] = nc.dram_tensor("sgw", [WIN, 128, 32], bf16, kind="ExternalInput")
    wshapes = dict(
        kvslab0=[6, KVROW], qcslab0=[6, QCW],
        kvslab1=[161, KVROW], qcslab1=[161, QCW],
        wep0=[20, HID], wep1=[20, HID], prep0=[128, HID], prep1=[128, HID],
        ident=[128, 128], wg1h1=[161, HID], wg1h2=[161, HID],
        wg2rep=[128, HID], bg2rep=[128, 1], wh1=[321, 320], wh2=[321, 6],
    )
    for k, shp in wshapes.items():
        d[k] = nc.dram_tensor(k, shp, bf16, kind="ExternalInput")
    out_d = nc.dram_tensor("out", [32, 6], f32, kind="ExternalOutput")
    dbg_d = nc.dram_tensor("dbg", [128, KVROW], f32, kind="ExternalOutput")

    kv_own = [nc.dram_tensor(f"kv_own{l}", [NSHARD, KVROW], bf16) for l in range(2)]
    kv_full = [nc.dram_tensor(f"kv_full{l}", [NPAD, KVROW], bf16, addr_space="Shared")
               for l in range(2)]
    hT = [None, nc.dram_tensor("hT1", [HID, NSHARD], bf16),
          nc.dram_tensor("hT2", [HID, NSHARD], bf16)]
    h_nm = [None, nc.dram_tensor("h_nm1", [NSHARD, HID], bf16),
            nc.dram_tensor("h_nm2", [NSHARD, HID], bf16)]
    pool_in = nc.dram_tensor("pool_in", [32, 321], f32)
    pool_out = nc.dram_tensor("pool_out", [32, 321], f32, addr_space="Shared")
    rg = [list(range(NCORES))]

    with tile.TileContext(nc) as tc:
        with (
            tc.tile_pool(name="const", bufs=1) as cst,
            tc.tile_pool(name="sb", bufs=2) as sb,
            tc.tile_pool(name="gath", bufs=4) as gath,
            tc.tile_pool(name="ps", bufs=2, space="PSUM") as ps,
        ):
            nc.gpsimd.load_library(mlp)
            regGS_L = nc.gpsimd.to_reg(GS_L)
            regGS_H = nc.gpsimd.to_reg(GS_H)

            C = {}
            def _load_const(key, part, cols, row0=0, dt=bf16):
                t = cst.tile([part, cols], dt, name=f"c_{key}_{row0}")
                nc.sync.dma_start(out=t[:], in_=d[key][row0:row0 + part, :])
                return t
            C["kvslab0"] = _load_const("kvslab0", 6, KVROW)
            C["qcslab0"] = _load_const("qcslab0", 6, QCW)
            C["kvslab1a"] = _load_const("kvslab1", 128, KVROW)
            C["kvslab1b"] = _load_const("kvslab1", 32, KVROW, 128)
            C["kvslab1c"] = _load_const("kvslab1", 1, KVROW, 160)
            C["qcslab1a"] = _load_const("qcslab1", 128, QCW)
            C["qcslab1b"] = _load_const("qcslab1", 32, QCW, 128)
            C["qcslab1c"] = _load_const("qcslab1", 1, QCW, 160)
            for l in range(2):
                C[f"wep{l}"] = _load_const(f"wep{l}", 20, HID)
                C[f"prep{l}"] = _load_const(f"prep{l}", 128, HID)
            C["ident"] = _load_const("ident", 128, 128)
            for key in ("wg1h1", "wg1h2"):
                C[key + "a"] = _load_const(key, 128, HID)
                C[key + "b"] = _load_const(key, 32, HID, 128)
            C["wg1bias"] = _load_const("wg1h1", 1, HID, 160)
            C["wg2rep"] = _load_const("wg2rep", 128, HID)
            C["bg2rep"] = _load_const("bg2rep", 128, 1)
            C["wh1a"] = _load_const("wh1", 128, 320)
            C["wh1b"] = _load_const("wh1", 128, 320, 128)
            C["wh1c"] = _load_const("wh1", 64, 320, 256)
            C["wh1d"] = _load_const("wh1", 1, 320, 320)
            C["wh2a"] = _load_const("wh2", 128, 6)
            C["wh2b"] = _load_const("wh2", 128, 6, 128)
            C["wh2c"] = _load_const("wh2", 64, 6, 256)
            C["wh2d"] = _load_const("wh2", 1, 6, 320)

            idxLt = cst.tile([128, NG * GS_L // 16], i16, name="idxLt")
            nc.sync.dma_start(out=idxLt[:], in_=d["idxL"][:])
            idxHt = cst.tile([128, NG * GS_H // 16], i16, name="idxHt")
            nc.sync.dma_start(out=idxHt[:], in_=d["idxH"][:])

            ones1 = cst.tile([1, 128], bf16, name="ones1")
            nc.gpsimd.memset(ones1[:], 1.0)
            eps4 = cst.tile([128, 4], f32, name="eps4")
            nc.gpsimd.memset(eps4[:], 1e-30)
            onep = cst.tile([128, 1], bf16, name="onep")
            nc.gpsimd.memset(onep[:], 1.0)
            eps32 = cst.tile([32, 1], f32, name="eps32")
            nc.gpsimd.memset(eps32[:], 1e-30)

            for layer in range(2):
                # ---- kv GEMM own nodes -> kv_own ----
                with nc.named_scope(f"kv{layer}"):
                    for t in range(WIN):
                        csl = slice(t * 128, (t + 1) * 128)
                        pkv = ps.tile([128, KVROW], f32, tag="kve", bufs=2)
                        if layer == 0:
                            xts = sb.tile([6, 128], bf16, tag="xts", bufs=3)
                            nc.sync.dma_start(out=xts[:], in_=d["x6T"][:, csl])
                            nc.tensor.matmul(pkv[:], xts[:], C["kvslab0"][:],
                                             start=True, stop=True)
                        else:
                            hta = sb.tile([128, 128], bf16, tag="hta", bufs=3)
                            nc.sync.dma_start(out=hta[:], in_=hT[1][0:128, csl])
                            htb = sb.tile([32, 128], bf16, tag="htb", bufs=3)
                            nc.sync.dma_start(out=htb[:], in_=hT[1][128:160, csl])
                            nc.tensor.matmul(pkv[:], hta[:], C["kvslab1a"][:], start=True, stop=False)
                            nc.tensor.matmul(pkv[:], htb[:], C["kvslab1b"][:], start=False, stop=False)
                            nc.tensor.matmul(pkv[:], ones1[:, :128], C["kvslab1c"][:], start=False, stop=True)
                        kvsb = sb.tile([128, KVROW], bf16, tag="kvsb")
                        nc.scalar.activation(out=kvsb[:], in_=pkv[:], func=AF.Copy)
                        nc.sync.dma_start(out=kv_own[layer][csl, :], in_=kvsb[:])
                with nc.named_scope(f"ag{layer}"):
                    nc.gpsimd.collective_compute(
                        "AllGather", mybir.AluOpType.bypass, replica_groups=rg,
                        ins=[kv_own[layer][0:3136, :]],
                        outs=[kv_full[layer][0:SPLIT, :]])
                    nc.gpsimd.collective_compute(
                        "AllGather", mybir.AluOpType.bypass, replica_groups=rg,
                        ins=[kv_own[layer][3136:NSHARD, :]],
                        outs=[kv_full[layer][SPLIT:NPAD, :]])

                # ---- edge phase ----
                with nc.named_scope(f"edge{layer}"):
                    cur = {"L": -1, "H": -1}
                    cur_tile = {"L": None, "H": None}

                    def _gather(region, gt):
                        if cur[region] == gt:
                            return cur_tile[region]
                        idxt, base, gsz, reg = (
                            (idxLt, 0, GS_L, regGS_L) if region == "L"
                            else (idxHt, SPLIT, GS_H, regGS_H)
                        )
                        gtile = gath.tile([128, gsz // 128, KVROW], bf16, tag="g" + region)
                        nc.gpsimd.dma_gather(
                            gtile[:],
                            kv_full[layer][base:base + SPLIT, :],
                            idxt[:, gt * (gsz // 16):(gt + 1) * (gsz // 16)],
                            num_idxs=gsz, num_idxs_reg=reg, elem_size=KVROW)
                        cur[region] = gt
                        cur_tile[region] = gtile
                        return gtile

                    for w in range(WIN):
                        wsl = slice(w * 128, (w + 1) * 128)
                        # window node GEMM -> q|C|r|-rQ
                        psq = ps.tile([128, QCW], f32, tag="pq", bufs=3)
                        if layer == 0:
                            xts = sb.tile([6, 128], bf16, tag="xts", bufs=3)
                            nc.sync.dma_start(out=xts[:], in_=d["x6T"][:, wsl])
                            nc.tensor.matmul(psq[:], xts[:], C["qcslab0"][:],
                                             start=True, stop=True)
                        else:
                            hta = sb.tile([128, 128], bf16, tag="hta", bufs=3)
                            nc.sync.dma_start(out=hta[:], in_=hT[1][0:128, wsl])
                            htb = sb.tile([32, 128], bf16, tag="htb", bufs=3)
                            nc.sync.dma_start(out=htb[:], in_=hT[1][128:160, wsl])
                            nc.tensor.matmul(psq[:], hta[:], C["qcslab1a"][:], start=True, stop=False)
                            nc.tensor.matmul(psq[:], htb[:], C["qcslab1b"][:], start=False, stop=False)
                            nc.tensor.matmul(psq[:], ones1[:, :128], C["qcslab1c"][:], start=False, stop=True)
                        qc = sb.tile([128, QCW], bf16, tag="qc", bufs=2)
                        nc.scalar.activation(out=qc[:], in_=psq[:], func=AF.Copy)

                        wst = sb.tile([128, NCH * 292], bf16, tag="wst", bufs=3)
                        nc.sync.dma_start(out=wst[:], in_=d["ws"][w])
                        stqt = wst[:, 0:NCH * 128]
                        stst = wst[:, NCH * 128:NCH * 256]
                        eact = wst[:, NCH * 256:NCH * 292].rearrange(
                            "p (j c) -> p j c", c=36)

                        gl = _gather("L", w)
                        gh = _gather("H", w)
                        halfL = 0
                        halfH = 0

                        # per-chunk qC one-hot gather matmuls (2 per PSUM bank)
                        qcg = sb.tile([128, NCH, QCROW], bf16, tag="qcg", bufs=2)
                        for pj in range((NCH + 1) // 2):
                            jn = min(2, NCH - pj * 2)
                            pq = ps.tile([128, 2, QCROW], f32, tag="pq", bufs=3)
                            for s in range(jn):
                                j = pj * 2 + s
                                nc.tensor.matmul(pq[:, s, :],
                                                 stqt[:, j * 128:(j + 1) * 128],
                                                 qc[:, 0:QCROW],
                                                 start=True, stop=True,
                                                 skip_group_check=True)
                            nc.scalar.activation(out=qcg[:, pj * 2:pj * 2 + jn, :],
                                                 in_=pq[:, 0:jn, :], func=AF.Copy)

                        # batched DVE per L/H group -- all plain 3D inner-contiguous
                        stage = sb.tile([128, NCH, 192], bf16, tag="stage", bufs=2)
                        al1 = sb.tile([128, NCH * 4], f32, tag="al1", bufs=2)
                        al2 = sb.tile([128, NCH * 4], f32, tag="al2", bufs=2)
                        al = sb.tile([128, NCH * 4], f32, tag="al", bufs=2)
                        wt = sb.tile([128, NCH, 192], bf16, tag="wt", bufs=2)
                        exg = sb.tile([128, NCH, 192], bf16, tag="exg", bufs=2)
                        for (g0, cnt, gt, half) in ((0, C_L, gl, halfL),
                                                    (C_L, C_H, gh, halfH)):
                            kvg = gt[:, half:half + cnt, :]
                            qs = qcg[:, g0:g0 + cnt, :]
                            # q*k -> stage[.., 0:160]
                            nc.vector.tensor_tensor(
                                out=stage[:, g0:g0 + cnt, 0:160],
                                in0=qs[:, :, 0:160],
                                in1=kvg[:, :, 0:160],
                                op=mybir.AluOpType.mult)
                            # ea*C -> stage[.., 160:176]
                            nc.vector.tensor_tensor(
                                out=stage[:, g0:g0 + cnt, 160:176],
                                in0=qs[:, :, 160:176],
                                in1=eact[:, g0:g0 + cnt, 0:16],
                                op=mybir.AluOpType.mult)
                        # alpha = sum_d q*k + sum_c ea*C (whole window)
                        nc.vector.tensor_reduce(
                            out=al1[:], in_=stage[:, :, 0:160]
                                .rearrange("p j (h dd) -> p j h dd", h=4),
                            axis=mybir.AxisListType.X, op=mybir.AluOpType.add)
                        nc.vector.tensor_reduce(
                            out=al2[:], in_=stage[:, :, 160:176]
                                .rearrange("p j (h c) -> p j h c", h=4),
                            axis=mybir.AxisListType.X, op=mybir.AluOpType.add)
                        nc.vector.tensor_add(al[:], al1[:], al2[:])
                        # exp-expand on ACT (broadcast input APs)
                        nc.scalar.activation(
                            out=exg[:, :, 0:160].rearrange("p j (h dd) -> p j h dd", h=4),
                            in_=al[:].rearrange("p (j h o) -> p j h o", h=4, o=1)
                                 .to_broadcast([128, NCH, 4, 40]),
                            func=AF.Exp, scale=INVSQD)
                        nc.scalar.activation(
                            out=exg[:, :, 160:180].rearrange("p j (c h) -> p j c h", c=5),
                            in_=al[:].rearrange("p (j o h) -> p j o h", o=1, h=4)
                                 .to_broadcast([128, NCH, 5, 4]),
                            func=AF.Exp, scale=INVSQD)
                        for (g0, cnt, gt, half) in ((0, C_L, gl, halfL),
                                                    (C_L, C_H, gh, halfH)):
                            kvg = gt[:, half:half + cnt, :]
                            # wt v-block = v_g * ex
                            nc.vector.tensor_tensor(
                                out=wt[:, g0:g0 + cnt, 0:160],
                                in0=kvg[:, :, 160:320],
                                in1=exg[:, g0:g0 + cnt, 0:160],
                                op=mybir.AluOpType.mult)
                            # wt S-block (c,h)-major incl ones col = ea|1 * ex
                            nc.vector.tensor_tensor(
                                out=wt[:, g0:g0 + cnt, 160:180],
                                in0=eact[:, g0:g0 + cnt, 16:36],
                                in1=exg[:, g0:g0 + cnt, 160:180],
                                op=mybir.AluOpType.mult)

                        # scatter: acc[nodes, (h,48)] += st^T @ wt
                        pacc = ps.tile([128, 192], f32, tag="acc", bufs=2)
                        for j in range(NCH):
                            nc.tensor.matmul(pacc[:],
                                             stst[:, j * 128:(j + 1) * 128],
                                             wt[:, j, :],
                                             start=(j == 0), stop=(j == NCH - 1),
                                             skip_group_check=True)

                        # ---- window post ----
                        accsb = sb.tile([128, 192], bf16, tag="accsb")
                        nc.scalar.activation(out=accsb[:], in_=pacc[:], func=AF.Copy)
                        # S correction: transpose accS [128, (c,h)] -> [20,128]
                        pst = ps.tile([20, 128], bf16, tag="tp", bufs=1)
                        nc.tensor.transpose(pst[:], accsb[:, 160:180], C["ident"][:])
                        tS = sb.tile([20, 128], bf16, tag="tS")
                        nc.scalar.activation(out=tS[:], in_=pst[:], func=AF.Copy)
                        pcorr = ps.tile([128, HID], f32, tag="tp", bufs=1)
                        nc.tensor.matmul(pcorr[:], tS[:], C[f"wep{layer}"][:],
                                         start=True, stop=True)
                        # outn = (accv + corr) * 1/denom
                        outn0 = sb.tile([128, HID], bf16, tag="outn0")
                        nc.vector.tensor_tensor(
                            out=outn0[:], in0=accsb[:, 0:160], in1=pcorr[:],
                            op=mybir.AluOpType.add)
                        dmax = sb.tile([128, 4], f32, tag="dmax")
                        nc.vector.tensor_tensor(out=dmax[:], in0=accsb[:, 176:180],
                                                in1=eps4[:], op=mybir.AluOpType.max)
                        denr = sb.tile([128, 4], f32, tag="denr")
                        nc.vector.reciprocal(out=denr[:], in_=dmax[:])
                        outn = sb.tile([128, HID], bf16, tag="outn")
                        nc.vector.tensor_tensor(
                            out=outn[:].rearrange("p (h dd) -> p h dd", h=4),
                            in0=outn0[:].rearrange("p (h dd) -> p h dd", h=4),
                            in1=denr[:].rearrange("p (h o) -> p h o", o=1)
                                .to_broadcast([128, 4, 40]),
                            op=mybir.AluOpType.mult)
                        # beta gate
                        scr = sb.tile([128, HID], bf16, tag="scr")
                        nc.vector.tensor_tensor(out=scr[:], in0=outn[:],
                                                in1=C[f"prep{layer}"][:],
                                                op=mybir.AluOpType.mult)
                        outP = sb.tile([128, 1], f32, tag="outP")
                        nc.vector.tensor_reduce(
                            out=outP[:], in_=scr[:].rearrange("p (a b) -> p a b", a=1),
                            axis=mybir.AxisListType.XY, op=mybir.AluOpType.add)
                        exb = sb.tile([128, 1], bf16, tag="exb")
                        nc.scalar.activation(out=exb[:], in_=outP[:], func=AF.Exp,
                                             scale=-1.0, bias=qc[:, 352:353])
                        betad = sb.tile([128, 1], bf16, tag="betad")
                        nc.vector.tensor_tensor(out=betad[:], in0=exb[:], in1=onep[:],
                                                op=mybir.AluOpType.add)
                        beta = sb.tile([128, 1], bf16, tag="beta")
                        with nc.allow_low_precision(reason="beta gate bf16 ok"):
                            nc.vector.reciprocal(out=beta[:], in_=betad[:])
                        dvec = sb.tile([128, HID], bf16, tag="dvec")
                        nc.vector.tensor_sub(dvec[:], qc[:, QCROW:QCROW + 160], outn[:])
                        hp = sb.tile([128, HID], bf16, tag="hp")
                        nc.vector.scalar_tensor_tensor(
                            out=hp[:], in0=dvec[:], scalar=beta[:, 0:1], in1=outn[:],
                            op0=mybir.AluOpType.mult, op1=mybir.AluOpType.add)
                        nc.sync.dma_start(out=h_nm[layer + 1][wsl, :], in_=hp[:])
                        ptr1 = ps.tile([128, 128], bf16, tag="tp", bufs=1)
                        nc.tensor.transpose(ptr1[:], hp[:, 0:128], C["ident"][:])
                        t1 = sb.tile([128, 128], bf16, tag="t1")
                        nc.scalar.activation(out=t1[:], in_=ptr1[:], func=AF.Copy)
                        nc.sync.dma_start(out=hT[layer + 1][0:128, wsl], in_=t1[:])
                        ptr2 = ps.tile([32, 128], bf16, tag="tp", bufs=1)
                        nc.tensor.transpose(ptr2[:], hp[:, 128:160], C["ident"][:])
                        t2 = sb.tile([32, 128], bf16, tag="t2")
                        nc.scalar.activation(out=t2[:], in_=ptr2[:], func=AF.Copy)
                        nc.sync.dma_start(out=hT[layer + 1][128:160, wsl], in_=t2[:])

            # ==== final phase: gate + graph pooling + head MLP ====
            with nc.named_scope("final"):
                pgr = ps.tile([32, 321], f32, tag="acc", bufs=2)
                for w in range(WIN):
                    wsl = slice(w * 128, (w + 1) * 128)
                    h1w = sb.tile([128, HID], bf16, tag="h1w")
                    nc.sync.dma_start(out=h1w[:], in_=h_nm[1][wsl, :])
                    h2w = sb.tile([128, HID], bf16, tag="h2w")
                    nc.sync.dma_start(out=h2w[:], in_=h_nm[2][wsl, :])
                    sgt = sb.tile([128, 32], bf16, tag="sgt", bufs=3)
                    nc.sync.dma_start(out=sgt[:], in_=d["sgw"][w])
                    pg = ps.tile([128, HID], f32, tag="kve", bufs=2)
                    first = True
                    for (src_hT, wkey) in ((hT[1], "wg1h1"), (hT[2], "wg1h2")):
                        g_a = sb.tile([128, 128], bf16, tag="hta", bufs=3)
                        nc.sync.dma_start(out=g_a[:], in_=src_hT[0:128, wsl])
                        g_b = sb.tile([32, 128], bf16, tag="htb", bufs=3)
                        nc.sync.dma_start(out=g_b[:], in_=src_hT[128:160, wsl])
                        nc.tensor.matmul(pg[:], g_a[:], C[wkey + "a"][:], start=first, stop=False)
                        first = False
                        nc.tensor.matmul(pg[:], g_b[:], C[wkey + "b"][:], start=False, stop=False)
                    nc.tensor.matmul(pg[:], ones1[:, :128], C["wg1bias"][:], start=False, stop=True)
                    grelu = sb.tile([128, HID], bf16, tag="grelu")
                    nc.scalar.activation(out=grelu[:], in_=pg[:], func=AF.Relu)
                    scr2 = sb.tile([128, HID], bf16, tag="scr")
                    gatec = sb.tile([128, 1], f32, tag="gatec")
                    nc.vector.tensor_tensor(out=scr2[:], in0=grelu[:],
                                            in1=C["wg2rep"][:], op=mybir.AluOpType.mult)
                    nc.vector.tensor_reduce(
                        out=gatec[:], in_=scr2[:].rearrange("p (a b) -> p a b", a=1),
                        axis=mybir.AxisListType.XY, op=mybir.AluOpType.add)
                    ge = sb.tile([128, 1], f32, tag="ge")
                    nc.scalar.activation(out=ge[:], in_=gatec[:], func=AF.Exp,
                                         bias=C["bg2rep"][:, 0:1])
                    wg = sb.tile([128, 321], bf16, tag="wg")
                    nc.vector.tensor_tensor(
                        out=wg[:, 0:HID], in0=h1w[:],
                        in1=ge[:].to_broadcast([128, HID]), op=mybir.AluOpType.mult)
                    nc.vector.tensor_tensor(
                        out=wg[:, HID:2 * HID], in0=h2w[:],
                        in1=ge[:].to_broadcast([128, HID]), op=mybir.AluOpType.mult)
                    nc.vector.tensor_copy(out=wg[:, 320:321], in_=ge[:])
                    nc.tensor.matmul(pgr[:], sgt[:], wg[:], start=(w == 0),
                                     stop=(w == WIN - 1), skip_group_check=True)
                pg_sb = sb.tile([32, 321], f32, tag="pg_sb")
                nc.vector.tensor_copy(out=pg_sb[:], in_=pgr[:])
                nc.sync.dma_start(out=pool_in[:], in_=pg_sb[:])
                nc.gpsimd.collective_compute(
                    "AllReduce", mybir.AluOpType.add, replica_groups=rg,
                    ins=[pool_in[:]], outs=[pool_out[:]])
                psb = sb.tile([32, 321], f32, tag="psb")
                nc.sync.dma_start(out=psb[:], in_=pool_out[:])
                gden = sb.tile([32, 1], f32, tag="gden")
                nc.vector.tensor_tensor(out=gden[:], in0=psb[:, 320:321],
                                        in1=eps32[:], op=mybir.AluOpType.max)
                gdr = sb.tile([32, 1], f32, tag="gdr")
                nc.vector.reciprocal(out=gdr[:], in_=gden[:])
                pl = sb.tile([32, 320], bf16, tag="pl")
                nc.vector.tensor_tensor(
                    out=pl[:], in0=psb[:, 0:320],
                    in1=gdr[:].to_broadcast([32, 320]), op=mybir.AluOpType.mult)

                def _headmm(vin, wa, wb, wc, wd, nout, tagp):
                    pouts = ps.tile([32, nout], f32, tag=tagp, bufs=(3 if tagp == "pq" else 2))
                    for si, (c0, m) in enumerate(((0, 128), (128, 128), (256, 64))):
                        ptt = ps.tile([m, 32], bf16, tag="tp", bufs=1)
                        nc.tensor.transpose(ptt[:], vin[:, c0:c0 + m], C["ident"][0:32, 0:32])
                        tsb = sb.tile([m, 32], bf16, tag="tsb")
                        nc.vector.tensor_copy(out=tsb[:], in_=ptt[:])
                        nc.tensor.matmul(pouts[:], tsb[:], (wa, wb, wc)[si][:m, :],
                                         start=(si == 0), stop=False, skip_group_check=True)
                    nc.tensor.matmul(pouts[:], ones1[:, :32], wd[:],
                                     start=False, stop=True, skip_group_check=True)
                    return pouts

                ph1 = _headmm(pl, C["wh1a"], C["wh1b"], C["wh1c"], C["wh1d"], 320, "pq")
                vrel = sb.tile([32, 320], bf16, tag="vrel")
                nc.scalar.activation(out=vrel[:], in_=ph1[:], func=AF.Relu)
                ph2 = _headmm(vrel, C["wh2a"], C["wh2b"], C["wh2c"], C["wh2d"], 6, "kve")
                osb = sb.tile([32, 6], f32, tag="osb")
                nc.vector.tensor_copy(out=osb[:], in_=ph2[:])
                nc.sync.dma_start(out=out_d[:], in_=osb[:])
                dbgt = sb.tile([128, KVROW], f32, tag="dbgt")
                nc.gpsimd.memset(dbgt[:], 0.0)
                nc.sync.dma_start(out=dbg_d[:], in_=dbgt[:])

    nc.compile()
    return nc


_CACHE = {}
_LAST_RES = None


def kernel(**inputs):
    inputs = {k: np.asarray(v) for k, v in inputs.items()}
    per_core, C_L, C_H = _preprocess(
        inputs["x"], inputs["edge_index"], inputs["edge_attr"], inputs["batch"])
    w = _weights(inputs)
    key = (C_L, C_H)
    if key not in _CACHE:
        _CACHE[key] = _build(C_L, C_H)
    nc = _CACHE[key]
    in_maps = []
    for r in range(NCORES):
        m = dict(w)
        m.update(per_core[r])
        in_maps.append(m)
    import os
    trace = bool(os.environ.get("KERNEL_TRACE"))
    if trace:
        try:
            import axon_prof
            axon_prof.install()
        except Exception:
            trace = False
    res = run_bass_kernel_spmd(nc, in_maps, core_ids=list(range(NCORES)), trace=trace)
    if trace and res.exec_time_ns is not None:
        print(f"HW exec time: {res.exec_time_ns} ns")
        if res.per_core_scope_times:
            for scope, cores in sorted(res.per_core_scope_times.items()):
                print(f"  scope {scope}: {cores}")
    global _LAST_RES
    _LAST_RES = res
    out = res.results[0]["out"]
    return out.reshape(G, 2, 3).astype(np.float32)


# revision 32
# speedup vs baseline: 1.1363x; 1.0101x over previous
"""Trainium2 Bass kernel for nn_EndpointRegressor (2x TransformerConv GNN +
AttentionalAggregation) distributed over 8 NeuronCores.  v2: bf16 datapath.

Sharding: edges partitioned by destination node range (6272 nodes/core);
each core owns its dst nodes exclusively, so segment softmax/scatter stats
need no cross-core reduction.  Per layer each core computes its nodes'
k|v table (384-col bf16 rows, biases + edge bias folded), AllGathers it,
and dma_gathers rows for its edge shard.  The per-edge projection
e = edge_attr @ We is never materialized: its alpha contribution comes via
a node-level C table (C[n,h,c] = q[n,h]·We[c,h], gathered to edges through
the one-hot st_T matmul together with q), and its value contribution via
scattered stats S[n,h,c] = sum_e ex*ea_c followed by a per-window rank-16
correction matmul S @ WeP.  Segment softmax uses exp without max
subtraction (alpha ~ ±0.1 for this model family); the denominator is the
c=4 (ones) column of S.  One-hot scatter/gather matrices are host-built
and streamed as bf16; all matmuls are bf16 (FWL fast-weight-load active),
accumulation stays in fp32 PSUM.
"""
import math
import numpy as np
import ml_dtypes

import concourse.bass as bass
import concourse.bacc as bacc
import concourse.mybir as mybir
import concourse.tile as tile
from concourse._compat import get_trn_type
from concourse.bass_utils import run_bass_kernel_spmd
from concourse.library_config import mlp

# ---- problem constants ----
N, E, G = 50000, 500000, 32
H, D = 4, 40
HID = H * D            # 160
NCORES = 8
NSHARD = 6272          # 49*128 nodes per core
NPAD = NCORES * NSHARD # 50176
WIN = NSHARD // 128    # 49
SPLIT = NPAD // 2      # 25088 (int16 gather indices => 2 tables)
NG = WIN               # one gather per window per stream (num_idxs <= 1024!)
INVSQD = 1.0 / math.sqrt(float(D))

KVROW = 384            # [k 160 | v 160 | pad 64]
QCROW = 192            # [q 160 | C 20 | pad 12]
QCW = QCROW + 161      # + [r 160 | -rQ 1] = 353

f32 = mybir.dt.float32
bf16 = mybir.dt.bfloat16
i16 = mybir.dt.int16
npbf = ml_dtypes.bfloat16

AF = mybir.ActivationFunctionType


def _wrap16(ix):
    """[n] int16 -> [128, n//16] dma_gather index layout (16-wrap, x8 replicate)."""
    return np.tile(ix.reshape(-1, 16).T, (8, 1))


def _preprocess(x, edge_index, edge_attr, batch):
    src = np.asarray(edge_index[0], dtype=np.int64)
    dst = np.asarray(edge_index[1], dtype=np.int64)
    ea = np.asarray(edge_attr, dtype=np.float32)
    order = np.argsort(dst, kind="stable")
    src, dst, ea = src[order], dst[order], ea[order]

    core = dst // NSHARD
    win = (dst % NSHARD) // 128
    # 4-quarter interleaved kv table (matches the 4 split AllGathers):
    # row(core r, local p) = (p//1568)*12544 + r*1568 + p%1568
    # L table = quarters 0,1 (p < 3136) -> rows 0:25088
    _p = src % NSHARD
    _r = src // NSHARD
    srow = (_p // 1568) * 12544 + _r * 1568 + (_p % 1568)
    low = srow < SPLIT

    buckets = {}
    for r in range(NCORES):
        m_r = core == r
        for w in range(WIN):
            m = m_r & (win == w)
            idx = np.nonzero(m)[0]
            buckets[(r, w)] = (idx[low[idx]], idx[~low[idx]])

    C_L = max(1, max((len(b[0]) + 127) // 128 for b in buckets.values()))
    C_H = max(1, max((len(b[1]) + 127) // 128 for b in buckets.values()))
    NCH = C_L + C_H
    GS_L, GS_H = C_L * 128, C_H * 128

    per_core = []
    for r in range(NCORES):
        Lslots = np.zeros(NG * GS_L, np.int64)
        Hslots = np.zeros(NG * GS_H, np.int64)
        eaC = np.zeros((WIN, 128, NCH, 36), np.float32)
        stq = np.zeros((WIN, 128, NCH * 128), np.float32)  # st_T [node, (chunk, edge)]
        sts = np.zeros((WIN, 128, NCH * 128), np.float32)  # st [edge, (chunk, node)]
        for w in range(WIN):
            lo, hi = buckets[(r, w)]
            for (idx_e, slots, Cg, j0, table_off) in (
                (lo, Lslots, C_L, 0, 0),
                (hi, Hslots, C_H, C_L, SPLIT),
            ):
                n = len(idx_e)
                s0 = w * Cg * 128
                slots[s0:s0 + n] = srow[idx_e] - table_off
                kk = np.arange(n)
                jj = j0 + kk // 128
                pp = kk % 128
                dr = (dst[idx_e] % 128).astype(np.int64)
                # cols 0:16 (h,c)-major; 16:32 (c,h)-major; 32:36 ones (c=4)
                for h in range(4):
                    eaC[w, pp, jj, h * 4:h * 4 + 4] = ea[idx_e]
                    eaC[w, pp, jj, 32 + h] = 1.0
                eaC[w, pp, jj, 16:32] = np.repeat(ea[idx_e], 4, axis=-1).reshape(-1, 16)
                stq[w, dr, jj * 128 + pp] = 1.0
                sts[w, pp, jj * 128 + dr] = 1.0
        # own-node arrays
        n0 = r * NSHARD
        x6T = np.zeros((6, NSHARD), np.float32)
        x6T[5, :] = 1.0
        sgw = np.zeros((WIN, 128, 32), np.float32)
        n_real = max(0, min(NSHARD, N - n0))
        if n_real > 0:
            x6T[:5, :n_real] = np.asarray(x[n0:n0 + n_real], np.float32).T
            bc = np.asarray(batch[n0:n0 + n_real], np.int64)
            sgw.reshape(NSHARD, 32)[np.arange(n_real), bc] = 1.0
        ws = np.concatenate(
            [stq, sts, eaC.reshape(WIN, 128, NCH * 36)], axis=-1)
        per_core.append(
            dict(
                x6T=x6T.astype(npbf),
                idxL=np.ascontiguousarray(_wrap16(Lslots.astype(np.int16))),
                idxH=np.ascontiguousarray(_wrap16(Hslots.astype(np.int16))),
                ws=np.ascontiguousarray(ws).astype(npbf),
                sgw=sgw.astype(npbf),
            )
        )
    return per_core, C_L, C_H


def _weights(inp):
    """Host-side weight packing (f64 folds -> bf16)."""
    w = {}
    W_in = np.asarray(inp["W_in"], np.float64)
    b_in = np.asarray(inp["b_in"], np.float64)
    for l in range(2):
        Wq, bq = inp["Wq"][l].astype(np.float64), inp["bq"][l].astype(np.float64)
        Wk, bk = inp["Wk"][l].astype(np.float64), inp["bk"][l].astype(np.float64)
        Wv, bv = inp["Wv"][l].astype(np.float64), inp["bv"][l].astype(np.float64)
        We, be = inp["We"][l].astype(np.float64), inp["be"][l].astype(np.float64)
        Wskip, bskip = inp["Wskip"][l].astype(np.float64), inp["bskip"][l].astype(np.float64)
        Wbeta = inp["Wbeta"][l].astype(np.float64)
        P = Wbeta[:HID, 0] + Wbeta[2 * HID:, 0]
        Q = Wbeta[HID:2 * HID, 0] - Wbeta[2 * HID:, 0]
        # WeP [20, 160]: rows (c, h) c<4 -> We[c, h-block]; c=4 rows zero
        WeP = np.zeros((20, HID), np.float64)
        WeC = np.zeros((HID, 16), np.float64)
        for h in range(H):
            for c in range(4):
                WeP[c * 4 + h, h * D:(h + 1) * D] = We[c, h * D:(h + 1) * D]
                WeC[h * D:(h + 1) * D, h * 4 + c] = We[c, h * D:(h + 1) * D]
        if l == 0:
            Wq_e = W_in @ Wq; bq_e = b_in @ Wq + bq
            Wk_e = W_in @ Wk; bk_e = b_in @ Wk + bk + be
            Wv_e = W_in @ Wv; bv_e = b_in @ Wv + bv + be
            Ws_e = W_in @ Wskip; bs_e = b_in @ Wskip + bskip
        else:
            Wq_e, bq_e = Wq, bq
            Wk_e, bk_e = Wk, bk + be
            Wv_e, bv_e = Wv, bv + be
            Ws_e, bs_e = Wskip, bskip
        nin = Wq_e.shape[0]
        kv_slab = np.zeros((nin + 1, KVROW), np.float64)
        kv_slab[:nin, 0:160] = Wk_e
        kv_slab[nin, 0:160] = bk_e
        kv_slab[:nin, 160:320] = Wv_e
        kv_slab[nin, 160:320] = bv_e
        qc_slab = np.zeros((nin + 1, QCW), np.float64)
        qc_slab[:nin, 0:160] = Wq_e
        qc_slab[nin, 0:160] = bq_e
        qc_slab[:nin, 160:176] = Wq_e @ WeC
        qc_slab[nin, 160:176] = bq_e @ WeC
        qc_slab[:nin, QCROW:QCROW + 160] = Ws_e
        qc_slab[nin, QCROW:QCROW + 160] = bs_e
        qc_slab[:nin, QCROW + 160] = -(Ws_e @ Q)
        qc_slab[nin, QCROW + 160] = -(bs_e @ Q)
        if l == 0:
            w["kvslab0"] = kv_slab.astype(npbf)       # [6, 384]
            w["qcslab0"] = qc_slab.astype(npbf)       # [6, 353]
        else:
            w["kvslab1"] = kv_slab.astype(npbf)       # [161, 384]
            w["qcslab1"] = qc_slab.astype(npbf)       # [161, 353]
        w[f"wep{l}"] = WeP.astype(npbf)               # [16, 160]
        w[f"prep{l}"] = np.broadcast_to(P, (128, HID)).astype(npbf).copy()
    w["ident"] = np.eye(128).astype(npbf)
    Wg1 = np.asarray(inp["Wg1"], np.float64)
    w["wg1h1"] = np.concatenate([Wg1[:HID], np.asarray(inp["bg1"], np.float64)[None, :]], 0).astype(npbf)  # [161,160]
    w["wg1h2"] = np.concatenate([Wg1[HID:], np.zeros((1, HID))], 0).astype(npbf)
    w["wg2rep"] = np.broadcast_to(np.asarray(inp["Wg2"], np.float64)[:, 0], (128, HID)).astype(npbf).copy()
    w["bg2rep"] = np.full((128, 1), float(np.asarray(inp["bg2"]).reshape(-1)[0])).astype(npbf)
    w["wh1"] = np.concatenate([np.asarray(inp["Wh1"], np.float64),
                               np.asarray(inp["bh1"], np.float64)[None, :]], 0).astype(npbf)  # [321,320]
    w["wh2"] = np.concatenate([np.asarray(inp["Wh2"], np.float64),
                               np.asarray(inp["bh2"], np.float64)[None, :]], 0).astype(npbf)  # [321,6]
    return w


def _build(C_L, C_H):
    NCH = C_L + C_H
    GS_L, GS_H = C_L * 128, C_H * 128
    assert GS_L <= 1024 and GS_H <= 1024, 'dma_gather num_idxs must be <= 1024'

    nc = bacc.Bacc(get_trn_type() or "TRN2", target_bir_lowering=False)

    d = {}
    d["x6T"] = nc.dram_tensor("x6T", [6, NSHARD], bf16, kind="ExternalInput")
    d["idxL"] = nc.dram_tensor("idxL", [128, NG * GS_L // 16], i16, kind="ExternalInput")
    d["idxH"] = nc.dram_tensor("idxH", [128, NG * GS_H // 16], i16, kind="ExternalInput")
    d["ws"] = nc.dram_tensor("ws", [WIN, 128, NCH * 292], bf16, kind="ExternalInput")
    d["sgw"] = nc.dram_tensor("sgw", [WIN, 128, 32], bf16, kind="ExternalInput")
    wshapes = dict(
        kvslab0=[6, KVROW], qcslab0=[6, QCW],
        kvslab1=[161, KVROW], qcslab1=[161, QCW],
        wep0=[20, HID], wep1=[20, HID], prep0=[128, HID], prep1=[128, HID],
        ident=[128, 128], wg1h1=[161, HID], wg1h2=[161, HID],
        wg2rep=[128, HID], bg2rep=[128, 1], wh1=[321, 320], wh2=[321, 6],
    )
    for k, shp in wshapes.items():
        d[k] = nc.dram_tensor(k, shp, bf16, kind="ExternalInput")
    out_d = nc.dram_tensor("out", [32, 6], f32, kind="ExternalOutput")
    dbg_d = nc.dram_tensor("dbg", [128, KVROW], f32, kind="ExternalOutput")

    kv_own = [nc.dram_tensor(f"kv_own{l}", [NSHARD, KVROW], bf16) for l in range(2)]
    kv_full = [nc.dram_tensor(f"kv_full{l}", [NPAD, KVROW], bf16, addr_space="Shared")
               for l in range(2)]
    hT = [None, nc.dram_tensor("hT1", [HID, NSHARD], bf16),
          nc.dram_tensor("hT2", [HID, NSHARD], bf16)]
    h_nm = [None, nc.dram_tensor("h_nm1", [NSHARD, HID], bf16),
            nc.dram_tensor("h_nm2", [NSHARD, HID], bf16)]
    pool_in = nc.dram_tensor("pool_in", [32, 321], f32)
    pool_out = nc.dram_tensor("pool_out", [32, 321], f32, addr_space="Shared")
    rg = [list(range(NCORES))]

    with tile.TileContext(nc) as tc:
        with (
            tc.tile_pool(name="const", bufs=1) as cst,
            tc.tile_pool(name="sb", bufs=2) as sb,
            tc.tile_pool(name="gath", bufs=4) as gath,
            tc.tile_pool(name="ps", bufs=2, space="PSUM") as ps,
        ):
            nc.gpsimd.load_library(mlp)
            regGS_L = nc.gpsimd.to_reg(GS_L)
            regGS_H = nc.gpsimd.to_reg(GS_H)

            C = {}
            def _load_const(key, part, cols, row0=0, dt=bf16):
                t = cst.tile([part, cols], dt, name=f"c_{key}_{row0}")
                nc.sync.dma_start(out=t[:], in_=d[key][row0:row0 + part, :])
                return t
            C["kvslab0"] = _load_const("kvslab0", 6, KVROW)
            C["qcslab0"] = _load_const("qcslab0", 6, QCW)
            C["kvslab1a"] = _load_const("kvslab1", 128, KVROW)
            C["kvslab1b"] = _load_const("kvslab1", 32, KVROW, 128)
            C["kvslab1c"] = _load_const("kvslab1", 1, KVROW, 160)
            C["qcslab1a"] = _load_const("qcslab1", 128, QCW)
            C["qcslab1b"] = _load_const("qcslab1", 32, QCW, 128)
            C["qcslab1c"] = _load_const("qcslab1", 1, QCW, 160)
            for l in range(2):
                C[f"wep{l}"] = _load_const(f"wep{l}", 20, HID)
                C[f"prep{l}"] = _load_const(f"prep{l}", 128, HID)
            C["ident"] = _load_const("ident", 128, 128)
            for key in ("wg1h1", "wg1h2"):
                C[key + "a"] = _load_const(key, 128, HID)
                C[key + "b"] = _load_const(key, 32, HID, 128)
            C["wg1bias"] = _load_const("wg1h1", 1, HID, 160)
            C["wg2rep"] = _load_const("wg2rep", 128, HID)
            C["bg2rep"] = _load_const("bg2rep", 128, 1)
            C["wh1a"] = _load_const("wh1", 128, 320)
            C["wh1b"] = _load_const("wh1", 128, 320, 128)
            C["wh1c"] = _load_const("wh1", 64, 320, 256)
            C["wh1d"] = _load_const("wh1", 1, 320, 320)
            C["wh2a"] = _load_const("wh2", 128, 6)
            C["wh2b"] = _load_const("wh2", 128, 6, 128)
            C["wh2c"] = _load_const("wh2", 64, 6, 256)
            C["wh2d"] = _load_const("wh2", 1, 6, 320)

            idxLt = cst.tile([128, NG * GS_L // 16], i16, name="idxLt")
            nc.sync.dma_start(out=idxLt[:], in_=d["idxL"][:])
            idxHt = cst.tile([128, NG * GS_H // 16], i16, name="idxHt")
            nc.sync.dma_start(out=idxHt[:], in_=d["idxH"][:])

            ones1 = cst.tile([1, 128], bf16, name="ones1")
            nc.gpsimd.memset(ones1[:], 1.0)
            eps4 = cst.tile([128, 4], f32, name="eps4")
            nc.gpsimd.memset(eps4[:], 1e-30)
            onep = cst.tile([128, 1], bf16, name="onep")
            nc.gpsimd.memset(onep[:], 1.0)
            eps32 = cst.tile([32, 1], f32, name="eps32")
            nc.gpsimd.memset(eps32[:], 1e-30)

            for layer in range(2):
                # ---- kv GEMM own nodes -> kv_own ----
                with nc.named_scope(f"kv{layer}"):
                    for t in range(WIN):
                        csl = slice(t * 128, (t + 1) * 128)
                        pkv = ps.tile([128, KVROW], f32, tag="kve", bufs=2)
                        if layer == 0:
                            xts = sb.tile([6, 128], bf16, tag="xts", bufs=3)
                            nc.sync.dma_start(out=xts[:], in_=d["x6T"][:, csl])
                            nc.tensor.matmul(pkv[:], xts[:], C["kvslab0"][:],
                                             start=True, stop=True)
                        else:
                            hta = sb.tile([128, 128], bf16, tag="hta", bufs=3)
                            nc.sync.dma_start(out=hta[:], in_=hT[1][0:128, csl])
                            htb = sb.tile([32, 128], bf16, tag="htb", bufs=3)
                            nc.sync.dma_start(out=htb[:], in_=hT[1][128:160, csl])
                            nc.tensor.matmul(pkv[:], hta[:], C["kvslab1a"][:], start=True, stop=False)
                            nc.tensor.matmul(pkv[:], htb[:], C["kvslab1b"][:], start=False, stop=False)
                            nc.tensor.matmul(pkv[:], ones1[:, :128], C["kvslab1c"][:], start=False, stop=True)
                        kvsb = sb.tile([128, KVROW], bf16, tag="kvsb")
                        nc.scalar.activation(out=kvsb[:], in_=pkv[:], func=AF.Copy)
                        nc.sync.dma_start(out=kv_own[layer][csl, :], in_=kvsb[:])
                with nc.named_scope(f"ag{layer}"):
                    for qi in range(4):
                        nc.gpsimd.collective_compute(
                            "AllGather", mybir.AluOpType.bypass, replica_groups=rg,
                            ins=[kv_own[layer][qi * 1568:(qi + 1) * 1568, :]],
                            outs=[kv_full[layer][qi * 12544:(qi + 1) * 12544, :]])

                # ---- edge phase ----
                with nc.named_scope(f"edge{layer}"):
                    cur = {"L": -1, "H": -1}
                    cur_tile = {"L": None, "H": None}

                    def _gather(region, gt):
                        if cur[region] == gt:
                            return cur_tile[region]
                        idxt, base, gsz, reg = (
                            (idxLt, 0, GS_L, regGS_L) if region == "L"
                            else (idxHt, SPLIT, GS_H, regGS_H)
                        )
                        gtile = gath.tile([128, gsz // 128, KVROW], bf16, tag="g" + region)
                        nc.gpsimd.dma_gather(
                            gtile[:],
                            kv_full[layer][base:base + SPLIT, :],
                            idxt[:, gt * (gsz // 16):(gt + 1) * (gsz // 16)],
                            num_idxs=gsz, num_idxs_reg=reg, elem_size=KVROW)
                        cur[region] = gt
                        cur_tile[region] = gtile
                        return gtile

                    for w in range(WIN):
                        wsl = slice(w * 128, (w + 1) * 128)
                        # window node GEMM -> q|C|r|-rQ
                        psq = ps.tile([128, QCW], f32, tag="pq", bufs=2)
                        if layer == 0:
                            xts = sb.tile([6, 128], bf16, tag="xts", bufs=3)
                            nc.sync.dma_start(out=xts[:], in_=d["x6T"][:, wsl])
                            nc.tensor.matmul(psq[:], xts[:], C["qcslab0"][:],
                                             start=True, stop=True)
                        else:
                            hta = sb.tile([128, 128], bf16, tag="hta", bufs=3)
                            nc.sync.dma_start(out=hta[:], in_=hT[1][0:128, wsl])
                            htb = sb.tile([32, 128], bf16, tag="htb", bufs=3)
                            nc.sync.dma_start(out=htb[:], in_=hT[1][128:160, wsl])
                            nc.tensor.matmul(psq[:], hta[:], C["qcslab1a"][:], start=True, stop=False)
                            nc.tensor.matmul(psq[:], htb[:], C["qcslab1b"][:], start=False, stop=False)
                            nc.tensor.matmul(psq[:], ones1[:, :128], C["qcslab1c"][:], start=False, stop=True)
                        qc = sb.tile([128, QCW], bf16, tag="qc", bufs=2)
                        nc.scalar.activation(out=qc[:], in_=psq[:], func=AF.Copy)

                        wst = sb.tile([128, NCH * 292], bf16, tag="wst", bufs=3)
                        nc.sync.dma_start(out=wst[:], in_=d["ws"][w])
                        stqt = wst[:, 0:NCH * 128]
                        stst = wst[:, NCH * 128:NCH * 256]
                        eact = wst[:, NCH * 256:NCH * 292].rearrange(
                            "p (j c) -> p j c", c=36)

                        gl = _gather("L", w)
                        gh = _gather("H", w)
                        halfL = 0
                        halfH = 0

                        # per-chunk qC one-hot gather matmuls (2 per PSUM bank)
                        qcg = sb.tile([128, NCH, QCROW], bf16, tag="qcg", bufs=2)
                        for pj in range((NCH + 1) // 2):
                            jn = min(2, NCH - pj * 2)
                            pq = ps.tile([128, 2, QCROW], f32, tag="pq", bufs=2)
                            for s in range(jn):
                                j = pj * 2 + s
                                nc.tensor.matmul(pq[:, s, :],
                                                 stqt[:, j * 128:(j + 1) * 128],
                                                 qc[:, 0:QCROW],
                                                 start=True, stop=True,
                                                 skip_group_check=True)
                            nc.scalar.activation(out=qcg[:, pj * 2:pj * 2 + jn, :],
                                                 in_=pq[:, 0:jn, :], func=AF.Copy)

                        # batched DVE per L/H group -- all plain 3D inner-contiguous
                        stage = sb.tile([128, NCH, 192], bf16, tag="stage", bufs=2)
                        al1 = sb.tile([128, NCH * 4], f32, tag="al1", bufs=2)
                        al2 = sb.tile([128, NCH * 4], f32, tag="al2", bufs=2)
                        al = sb.tile([128, NCH * 4], f32, tag="al", bufs=2)
                        wt = sb.tile([128, NCH, 192], bf16, tag="wt", bufs=2)
                        exg = sb.tile([128, NCH, 192], bf16, tag="exg", bufs=2)
                        for (g0, cnt, gt, half) in ((0, C_L, gl, halfL),
                                                    (C_L, C_H, gh, halfH)):
                            kvg = gt[:, half:half + cnt, :]
                            qs = qcg[:, g0:g0 + cnt, :]
                            # q*k -> stage[.., 0:160]
                            nc.vector.tensor_tensor(
                                out=stage[:, g0:g0 + cnt, 0:160],
                                in0=qs[:, :, 0:160],
                                in1=kvg[:, :, 0:160],
                                op=mybir.AluOpType.mult)
                            # ea*C -> stage[.., 160:176]
                            nc.vector.tensor_tensor(
                                out=stage[:, g0:g0 + cnt, 160:176],
                                in0=qs[:, :, 160:176],
                                in1=eact[:, g0:g0 + cnt, 0:16],
                                op=mybir.AluOpType.mult)
                        # alpha = sum_d q*k + sum_c ea*C (whole window)
                        nc.vector.tensor_reduce(
                            out=al1[:], in_=stage[:, :, 0:160]
                                .rearrange("p j (h dd) -> p j h dd", h=4),
                            axis=mybir.AxisListType.X, op=mybir.AluOpType.add)
                        nc.vector.tensor_reduce(
                            out=al2[:], in_=stage[:, :, 160:176]
                                .rearrange("p j (h c) -> p j h c", h=4),
                            axis=mybir.AxisListType.X, op=mybir.AluOpType.add)
                        nc.vector.tensor_add(al[:], al1[:], al2[:])
                        # exp-expand on ACT (broadcast input APs)
                        nc.scalar.activation(
                            out=exg[:, :, 0:160].rearrange("p j (h dd) -> p j h dd", h=4),
                            in_=al[:].rearrange("p (j h o) -> p j h o", h=4, o=1)
                                 .to_broadcast([128, NCH, 4, 40]),
                            func=AF.Exp, scale=INVSQD)
                        nc.scalar.activation(
                            out=exg[:, :, 160:180].rearrange("p j (c h) -> p j c h", c=5),
                            in_=al[:].rearrange("p (j o h) -> p j o h", o=1, h=4)
                                 .to_broadcast([128, NCH, 5, 4]),
                            func=AF.Exp, scale=INVSQD)
                        for (g0, cnt, gt, half) in ((0, C_L, gl, halfL),
                                                    (C_L, C_H, gh, halfH)):
                            kvg = gt[:, half:half + cnt, :]
                            # wt v-block = v_g * ex
                            nc.vector.tensor_tensor(
                                out=wt[:, g0:g0 + cnt, 0:160],
                                in0=kvg[:, :, 160:320],
                                in1=exg[:, g0:g0 + cnt, 0:160],
                                op=mybir.AluOpType.mult)
                            # wt S-block (c,h)-major incl ones col = ea|1 * ex
                            nc.vector.tensor_tensor(
                                out=wt[:, g0:g0 + cnt, 160:180],
                                in0=eact[:, g0:g0 + cnt, 16:36],
                                in1=exg[:, g0:g0 + cnt, 160:180],
                                op=mybir.AluOpType.mult)

                        # scatter: acc[nodes, (h,48)] += st^T @ wt
                        pacc = ps.tile([128, 192], f32, tag="acc", bufs=2)
                        for j in range(NCH):
                            nc.tensor.matmul(pacc[:],
                                             stst[:, j * 128:(j + 1) * 128],
                                             wt[:, j, :],
                                             start=(j == 0), stop=(j == NCH - 1),
                                             skip_group_check=True)

                        # ---- window post ----
                        accsb = sb.tile([128, 192], bf16, tag="accsb")
                        nc.scalar.activation(out=accsb[:], in_=pacc[:], func=AF.Copy)
                        # S correction: transpose accS [128, (c,h)] -> [20,128]
                        pst = ps.tile([20, 128], bf16, tag="tp", bufs=1)
                        nc.tensor.transpose(pst[:], accsb[:, 160:180], C["ident"][:])
                        tS = sb.tile([20, 128], bf16, tag="tS")
                        nc.scalar.activation(out=tS[:], in_=pst[:], func=AF.Copy)
                        pcorr = ps.tile([128, HID], f32, tag="tp", bufs=1)
                        nc.tensor.matmul(pcorr[:], tS[:], C[f"wep{layer}"][:],
                                         start=True, stop=True)
                        # outn = (accv + corr) * 1/denom
                        outn0 = sb.tile([128, HID], bf16, tag="outn0")
                        nc.vector.tensor_tensor(
                            out=outn0[:], in0=accsb[:, 0:160], in1=pcorr[:],
                            op=mybir.AluOpType.add)
                        dmax = sb.tile([128, 4], f32, tag="dmax")
                        nc.vector.tensor_tensor(out=dmax[:], in0=accsb[:, 176:180],
                                                in1=eps4[:], op=mybir.AluOpType.max)
                        denr = sb.tile([128, 4], f32, tag="denr")
                        nc.vector.reciprocal(out=denr[:], in_=dmax[:])
                        outn = sb.tile([128, HID], bf16, tag="outn")
                        nc.vector.tensor_tensor(
                            out=outn[:].rearrange("p (h dd) -> p h dd", h=4),
                            in0=outn0[:].rearrange("p (h dd) -> p h dd", h=4),
                            in1=denr[:].rearrange("p (h o) -> p h o", o=1)
                                .to_broadcast([128, 4, 40]),
                            op=mybir.AluOpType.mult)
                        # beta gate
                        scr = sb.tile([128, HID], bf16, tag="scr")
                        nc.vector.tensor_tensor(out=scr[:], in0=outn[:],
                                                in1=C[f"prep{layer}"][:],
                                                op=mybir.AluOpType.mult)
                        outP = sb.tile([128, 1], f32, tag="outP")
                        nc.vector.tensor_reduce(
                            out=outP[:], in_=scr[:].rearrange("p (a b) -> p a b", a=1),
                            axis=mybir.AxisListType.XY, op=mybir.AluOpType.add)
                        exb = sb.tile([128, 1], bf16, tag="exb")
                        nc.scalar.activation(out=exb[:], in_=outP[:], func=AF.Exp,
                                             scale=-1.0, bias=qc[:, 352:353])
                        betad = sb.tile([128, 1], bf16, tag="betad")
                        nc.vector.tensor_tensor(out=betad[:], in0=exb[:], in1=onep[:],
                                                op=mybir.AluOpType.add)
                        beta = sb.tile([128, 1], bf16, tag="beta")
                        with nc.allow_low_precision(reason="beta gate bf16 ok"):
                            nc.vector.reciprocal(out=beta[:], in_=betad[:])
                        dvec = sb.tile([128, HID], bf16, tag="dvec")
                        nc.vector.tensor_sub(dvec[:], qc[:, QCROW:QCROW + 160], outn[:])
                        hp = sb.tile([128, HID], bf16, tag="hp")
                        nc.vector.scalar_tensor_tensor(
                            out=hp[:], in0=dvec[:], scalar=beta[:, 0:1], in1=outn[:],
                            op0=mybir.AluOpType.mult, op1=mybir.AluOpType.add)
                        nc.sync.dma_start(out=h_nm[layer + 1][wsl, :], in_=hp[:])
                        ptr1 = ps.tile([128, 128], bf16, tag="tp", bufs=1)
                        nc.tensor.transpose(ptr1[:], hp[:, 0:128], C["ident"][:])
                        t1 = sb.tile([128, 128], bf16, tag="t1")
                        nc.scalar.activation(out=t1[:], in_=ptr1[:], func=AF.Copy)
                        nc.sync.dma_start(out=hT[layer + 1][0:128, wsl], in_=t1[:])
                        ptr2 = ps.tile([32, 128], bf16, tag="tp", bufs=1)
                        nc.tensor.transpose(ptr2[:], hp[:, 128:160], C["ident"][:])
                        t2 = sb.tile([32, 128], bf16, tag="t2")
                        nc.scalar.activation(out=t2[:], in_=ptr2[:], func=AF.Copy)
                        nc.sync.dma_start(out=hT[layer + 1][128:160, wsl], in_=t2[:])

            # ==== final phase: gate + graph pooling + head MLP ====
            with nc.named_scope("final"):
                pgr = ps.tile([32, 321], f32, tag="pgr", bufs=1)
                for w in range(WIN):
                    wsl = slice(w * 128, (w + 1) * 128)
                    h1w = sb.tile([128, HID], bf16, tag="h1w")
                    nc.sync.dma_start(out=h1w[:], in_=h_nm[1][wsl, :])
                    h2w = sb.tile([128, HID], bf16, tag="h2w")
                    nc.sync.dma_start(out=h2w[:], in_=h_nm[2][wsl, :])
                    sgt = sb.tile([128, 32], bf16, tag="sgt", bufs=3)
                    nc.sync.dma_start(out=sgt[:], in_=d["sgw"][w])
                    pg = ps.tile([128, HID], f32, tag="kve", bufs=2)
                    first = True
                    for (src_hT, wkey) in ((hT[1], "wg1h1"), (hT[2], "wg1h2")):
                        g_a = sb.tile([128, 128], bf16, tag="hta", bufs=3)
                        nc.sync.dma_start(out=g_a[:], in_=src_hT[0:128, wsl])
                        g_b = sb.tile([32, 128], bf16, tag="htb", bufs=3)
                        nc.sync.dma_start(out=g_b[:], in_=src_hT[128:160, wsl])
                        nc.tensor.matmul(pg[:], g_a[:], C[wkey + "a"][:], start=first, stop=False)
                        first = False
                        nc.tensor.matmul(pg[:], g_b[:], C[wkey + "b"][:], start=False, stop=False)
                    nc.tensor.matmul(pg[:], ones1[:, :128], C["wg1bias"][:], start=False, stop=True)
                    grelu = sb.tile([128, HID], bf16, tag="grelu")
                    nc.scalar.activation(out=grelu[:], in_=pg[:], func=AF.Relu)
                    scr2 = sb.tile([128, HID], bf16, tag="scr")
                    gatec = sb.tile([128, 1], f32, tag="gatec")
                    nc.vector.tensor_tensor(out=scr2[:], in0=grelu[:],
                                            in1=C["wg2rep"][:], op=mybir.AluOpType.mult)
                    nc.vector.tensor_reduce(
                        out=gatec[:], in_=scr2[:].rearrange("p (a b) -> p a b", a=1),
                        axis=mybir.AxisListType.XY, op=mybir.AluOpType.add)
                    ge = sb.tile([128, 1], f32, tag="ge")
                    nc.scalar.activation(out=ge[:], in_=gatec[:], func=AF.Exp,
                                         bias=C["bg2rep"][:, 0:1])
                    wg = sb.tile([128, 321], bf16, tag="wg")
                    nc.vector.tensor_tensor(
                        out=wg[:, 0:HID], in0=h1w[:],
                        in1=ge[:].to_broadcast([128, HID]), op=mybir.AluOpType.mult)
                    nc.vector.tensor_tensor(
                        out=wg[:, HID:2 * HID], in0=h2w[:],
                        in1=ge[:].to_broadcast([128, HID]), op=mybir.AluOpType.mult)
                    nc.vector.tensor_copy(out=wg[:, 320:321], in_=ge[:])
                    nc.tensor.matmul(pgr[:], sgt[:], wg[:], start=(w == 0),
                                     stop=(w == WIN - 1), skip_group_check=True)
                pg_sb = sb.tile([32, 321], f32, tag="pg_sb")
                nc.vector.tensor_copy(out=pg_sb[:], in_=pgr[:])
                nc.sync.dma_start(out=pool_in[:], in_=pg_sb[:])
                nc.gpsimd.collective_compute(
                    "AllReduce", mybir.AluOpType.add, replica_groups=rg,
                    ins=[pool_in[:]], outs=[pool_out[:]])
                psb = sb.tile([32, 321], f32, tag="psb")
                nc.sync.dma_start(out=psb[:], in_=pool_out[:])
                gden = sb.tile([32, 1], f32, tag="gden")
                nc.vector.tensor_tensor(out=gden[:], in0=psb[:, 320:321],
                                        in1=eps32[:], op=mybir.AluOpType.max)
                gdr = sb.tile([32, 1], f32, tag="gdr")
                nc.vector.reciprocal(out=gdr[:], in_=gden[:])
                pl = sb.tile([32, 320], bf16, tag="pl")
                nc.vector.tensor_tensor(
                    out=pl[:], in0=psb[:, 0:320],
                    in1=gdr[:].to_broadcast([32, 320]), op=mybir.AluOpType.mult)

                def _headmm(vin, wa, wb, wc, wd, nout, tagp):
                    pouts = ps.tile([32, nout], f32, tag=tagp, bufs=2)
                    for si, (c0, m) in enumerate(((0, 128), (128, 128), (256, 64))):
                        ptt = ps.tile([m, 32], bf16, tag="tp", bufs=1)
                        nc.tensor.transpose(ptt[:], vin[:, c0:c0 + m], C["ident"][0:32, 0:32])
                        tsb = sb.tile([m, 32], bf16, tag="tsb")
                        nc.vector.tensor_copy(out=tsb[:], in_=ptt[:])
                        nc.tensor.matmul(pouts[:], tsb[:], (wa, wb, wc)[si][:m, :],
                                         start=(si == 0), stop=False, skip_group_check=True)
                    nc.tensor.matmul(pouts[:], ones1[:, :32], wd[:],
                                     start=False, stop=True, skip_group_check=True)
                    return pouts

                ph1 = _headmm(pl, C["wh1a"], C["wh1b"], C["wh1c"], C["wh1d"], 320, "pq")
                vrel = sb.tile([32, 320], bf16, tag="vrel")
                nc.scalar.activation(out=vrel[:], in_=ph1[:], func=AF.Relu)
                ph2 = _headmm(vrel, C["wh2a"], C["wh2b"], C["wh2c"], C["wh2d"], 6, "kve")
                osb = sb.tile([32, 6], f32, tag="osb")
                nc.vector.tensor_copy(out=osb[:], in_=ph2[:])
                nc.sync.dma_start(out=out_d[:], in_=osb[:])
                dbgt = sb.tile([128, KVROW], f32, tag="dbgt")
                nc.gpsimd.memset(dbgt[:], 0.0)
                nc.sync.dma_start(out=dbg_d[:], in_=dbgt[:])

    nc.compile()
    return nc


_CACHE = {}
_LAST_RES = None


def kernel(**inputs):
    inputs = {k: np.asarray(v) for k, v in inputs.items()}
    per_core, C_L, C_H = _preprocess(
        inputs["x"], inputs["edge_index"], inputs["edge_attr"], inputs["batch"])
    w = _weights(inputs)
    key = (C_L, C_H)
    if key not in _CACHE:
        _CACHE[key] = _build(C_L, C_H)
    nc = _CACHE[key]
    in_maps = []
    for r in range(NCORES):
        m = dict(w)
        m.update(per_core[r])
        in_maps.append(m)
    import os
    trace = bool(os.environ.get("KERNEL_TRACE"))
    if trace:
        try:
            import axon_prof
            axon_prof.install()
        except Exception:
            trace = False
    res = run_bass_kernel_spmd(nc, in_maps, core_ids=list(range(NCORES)), trace=trace)
    if trace and res.exec_time_ns is not None:
        print(f"HW exec time: {res.exec_time_ns} ns")
        if res.per_core_scope_times:
            for scope, cores in sorted(res.per_core_scope_times.items()):
                print(f"  scope {scope}: {cores}")
    global _LAST_RES
    _LAST_RES = res
    out = res.results[0]["out"]
    return out.reshape(G, 2, 3).astype(np.float32)


# revision 33
# speedup vs baseline: 1.2036x; 1.0592x over previous
"""Trainium2 Bass kernel for nn_EndpointRegressor (2x TransformerConv GNN +
AttentionalAggregation) distributed over 8 NeuronCores.  v2: bf16 datapath.

Sharding: edges partitioned by destination node range (6272 nodes/core);
each core owns its dst nodes exclusively, so segment softmax/scatter stats
need no cross-core reduction.  Per layer each core computes its nodes'
k|v table (384-col bf16 rows, biases + edge bias folded), AllGathers it,
and dma_gathers rows for its edge shard.  The per-edge projection
e = edge_attr @ We is never materialized: its alpha contribution comes via
a node-level C table (C[n,h,c] = q[n,h]·We[c,h], gathered to edges through
the one-hot st_T matmul together with q), and its value contribution via
scattered stats S[n,h,c] = sum_e ex*ea_c followed by a per-window rank-16
correction matmul S @ WeP.  Segment softmax uses exp without max
subtraction (alpha ~ ±0.1 for this model family); the denominator is the
c=4 (ones) column of S.  One-hot scatter/gather matrices are host-built
and streamed as bf16; all matmuls are bf16 (FWL fast-weight-load active),
accumulation stays in fp32 PSUM.
"""
import math
import numpy as np
import ml_dtypes

import concourse.bass as bass
import concourse.bacc as bacc
import concourse.mybir as mybir
import concourse.tile as tile
from concourse._compat import get_trn_type
from concourse.bass_utils import run_bass_kernel_spmd
from concourse.library_config import mlp

# ---- problem constants ----
N, E, G = 50000, 500000, 32
H, D = 4, 40
HID = H * D            # 160
NCORES = 8
NSHARD = 6272          # 49*128 nodes per core
NPAD = NCORES * NSHARD # 50176
WIN = NSHARD // 128    # 49
SPLIT = NPAD // 2      # 25088 (int16 gather indices => 2 tables)
NG = WIN               # one gather per window per stream (num_idxs <= 1024!)
INVSQD = 1.0 / math.sqrt(float(D))

KVROW = 384            # [k 160 | v 160 | pad 64]
QCROW = 192            # [q 160 | C 20 | pad 12]
QCW = QCROW + 161      # + [r 160 | -rQ 1] = 353

f32 = mybir.dt.float32
bf16 = mybir.dt.bfloat16
i16 = mybir.dt.int16
npbf = ml_dtypes.bfloat16

AF = mybir.ActivationFunctionType


def _wrap16(ix):
    """[n] int16 -> [128, n//16] dma_gather index layout (16-wrap, x8 replicate)."""
    return np.tile(ix.reshape(-1, 16).T, (8, 1))


def _preprocess(x, edge_index, edge_attr, batch):
    src = np.asarray(edge_index[0], dtype=np.int64)
    dst = np.asarray(edge_index[1], dtype=np.int64)
    ea = np.asarray(edge_attr, dtype=np.float32)
    order = np.argsort(dst, kind="stable")
    src, dst, ea = src[order], dst[order], ea[order]

    core = dst // NSHARD
    win = (dst % NSHARD) // 128
    # 4-quarter interleaved kv table (matches the 4 split AllGathers):
    # row(core r, local p) = (p//1568)*12544 + r*1568 + p%1568
    # L table = quarters 0,1 (p < 3136) -> rows 0:25088
    _p = src % NSHARD
    _r = src // NSHARD
    srow = (_p // 1568) * 12544 + _r * 1568 + (_p % 1568)
    low = srow < SPLIT

    buckets = {}
    for r in range(NCORES):
        m_r = core == r
        for w in range(WIN):
            m = m_r & (win == w)
            idx = np.nonzero(m)[0]
            buckets[(r, w)] = (idx[low[idx]], idx[~low[idx]])

    C_L = max(1, max((len(b[0]) + 127) // 128 for b in buckets.values()))
    C_H = max(1, max((len(b[1]) + 127) // 128 for b in buckets.values()))
    NCH = C_L + C_H
    GS_L, GS_H = C_L * 128, C_H * 128

    per_core = []
    for r in range(NCORES):
        Lslots = np.zeros(NG * GS_L, np.int64)
        Hslots = np.zeros(NG * GS_H, np.int64)
        eaC = np.zeros((WIN, 128, NCH, 36), np.float32)
        stq = np.zeros((WIN, 128, NCH * 128), np.float32)  # st_T [node, (chunk, edge)]
        sts = np.zeros((WIN, 128, NCH * 128), np.float32)  # st [edge, (chunk, node)]
        for w in range(WIN):
            lo, hi = buckets[(r, w)]
            for (idx_e, slots, Cg, j0, table_off) in (
                (lo, Lslots, C_L, 0, 0),
                (hi, Hslots, C_H, C_L, SPLIT),
            ):
                n = len(idx_e)
                s0 = w * Cg * 128
                slots[s0:s0 + n] = srow[idx_e] - table_off
                kk = np.arange(n)
                jj = j0 + kk // 128
                pp = kk % 128
                dr = (dst[idx_e] % 128).astype(np.int64)
                # cols 0:16 (h,c)-major; 16:32 (c,h)-major; 32:36 ones (c=4)
                for h in range(4):
                    eaC[w, pp, jj, h * 4:h * 4 + 4] = ea[idx_e]
                    eaC[w, pp, jj, 32 + h] = 1.0
                eaC[w, pp, jj, 16:32] = np.repeat(ea[idx_e], 4, axis=-1).reshape(-1, 16)
                stq[w, dr, jj * 128 + pp] = 1.0
                sts[w, pp, jj * 128 + dr] = 1.0
        # own-node arrays
        n0 = r * NSHARD
        x6T = np.zeros((6, NSHARD), np.float32)
        x6T[5, :] = 1.0
        sgw = np.zeros((WIN, 128, 32), np.float32)
        n_real = max(0, min(NSHARD, N - n0))
        if n_real > 0:
            x6T[:5, :n_real] = np.asarray(x[n0:n0 + n_real], np.float32).T
            bc = np.asarray(batch[n0:n0 + n_real], np.int64)
            sgw.reshape(NSHARD, 32)[np.arange(n_real), bc] = 1.0
        ws = np.concatenate(
            [stq, sts, eaC.reshape(WIN, 128, NCH * 36)], axis=-1)
        per_core.append(
            dict(
                x6T=x6T.astype(npbf),
                idxL=np.ascontiguousarray(_wrap16(Lslots.astype(np.int16))),
                idxH=np.ascontiguousarray(_wrap16(Hslots.astype(np.int16))),
                ws=np.ascontiguousarray(ws).astype(npbf),
                sgw=sgw.astype(npbf),
            )
        )
    return per_core, C_L, C_H


def _weights(inp):
    """Host-side weight packing (f64 folds -> bf16)."""
    w = {}
    W_in = np.asarray(inp["W_in"], np.float64)
    b_in = np.asarray(inp["b_in"], np.float64)
    for l in range(2):
        Wq, bq = inp["Wq"][l].astype(np.float64), inp["bq"][l].astype(np.float64)
        Wk, bk = inp["Wk"][l].astype(np.float64), inp["bk"][l].astype(np.float64)
        Wv, bv = inp["Wv"][l].astype(np.float64), inp["bv"][l].astype(np.float64)
        We, be = inp["We"][l].astype(np.float64), inp["be"][l].astype(np.float64)
        Wskip, bskip = inp["Wskip"][l].astype(np.float64), inp["bskip"][l].astype(np.float64)
        Wbeta = inp["Wbeta"][l].astype(np.float64)
        P = Wbeta[:HID, 0] + Wbeta[2 * HID:, 0]
        Q = Wbeta[HID:2 * HID, 0] - Wbeta[2 * HID:, 0]
        # WeP [20, 160]: rows (c, h) c<4 -> We[c, h-block]; c=4 rows zero
        WeP = np.zeros((20, HID), np.float64)
        WeC = np.zeros((HID, 16), np.float64)
        for h in range(H):
            for c in range(4):
                WeP[c * 4 + h, h * D:(h + 1) * D] = We[c, h * D:(h + 1) * D]
                WeC[h * D:(h + 1) * D, h * 4 + c] = We[c, h * D:(h + 1) * D]
        if l == 0:
            Wq_e = W_in @ Wq; bq_e = b_in @ Wq + bq
            Wk_e = W_in @ Wk; bk_e = b_in @ Wk + bk + be
            Wv_e = W_in @ Wv; bv_e = b_in @ Wv + bv + be
            Ws_e = W_in @ Wskip; bs_e = b_in @ Wskip + bskip
        else:
            Wq_e, bq_e = Wq, bq
            Wk_e, bk_e = Wk, bk + be
            Wv_e, bv_e = Wv, bv + be
            Ws_e, bs_e = Wskip, bskip
        nin = Wq_e.shape[0]
        kv_slab = np.zeros((nin + 1, KVROW), np.float64)
        kv_slab[:nin, 0:160] = Wk_e
        kv_slab[nin, 0:160] = bk_e
        kv_slab[:nin, 160:320] = Wv_e
        kv_slab[nin, 160:320] = bv_e
        qc_slab = np.zeros((nin + 1, QCW), np.float64)
        qc_slab[:nin, 0:160] = Wq_e
        qc_slab[nin, 0:160] = bq_e
        qc_slab[:nin, 160:176] = Wq_e @ WeC
        qc_slab[nin, 160:176] = bq_e @ WeC
        qc_slab[:nin, QCROW:QCROW + 160] = Ws_e
        qc_slab[nin, QCROW:QCROW + 160] = bs_e
        qc_slab[:nin, QCROW + 160] = -(Ws_e @ Q)
        qc_slab[nin, QCROW + 160] = -(bs_e @ Q)
        if l == 0:
            w["kvslab0"] = kv_slab.astype(npbf)       # [6, 384]
            w["qcslab0"] = qc_slab.astype(npbf)       # [6, 353]
        else:
            w["kvslab1"] = kv_slab.astype(npbf)       # [161, 384]
            w["qcslab1"] = qc_slab.astype(npbf)       # [161, 353]
        w[f"wep{l}"] = WeP.astype(npbf)               # [16, 160]
        w[f"prep{l}"] = np.broadcast_to(P, (128, HID)).astype(npbf).copy()
    w["ident"] = np.eye(128).astype(npbf)
    Wg1 = np.asarray(inp["Wg1"], np.float64)
    w["wg1h1"] = np.concatenate([Wg1[:HID], np.asarray(inp["bg1"], np.float64)[None, :]], 0).astype(npbf)  # [161,160]
    w["wg1h2"] = np.concatenate([Wg1[HID:], np.zeros((1, HID))], 0).astype(npbf)
    w["wg2rep"] = np.broadcast_to(np.asarray(inp["Wg2"], np.float64)[:, 0], (128, HID)).astype(npbf).copy()
    w["bg2rep"] = np.full((128, 1), float(np.asarray(inp["bg2"]).reshape(-1)[0])).astype(npbf)
    w["wh1"] = np.concatenate([np.asarray(inp["Wh1"], np.float64),
                               np.asarray(inp["bh1"], np.float64)[None, :]], 0).astype(npbf)  # [321,320]
    w["wh2"] = np.concatenate([np.asarray(inp["Wh2"], np.float64),
                               np.asarray(inp["bh2"], np.float64)[None, :]], 0).astype(npbf)  # [321,6]
    return w


def _build(C_L, C_H):
    NCH = C_L + C_H
    GS_L, GS_H = C_L * 128, C_H * 128
    assert GS_L <= 1024 and GS_H <= 1024, 'dma_gather num_idxs must be <= 1024'

    nc = bacc.Bacc(get_trn_type() or "TRN2", target_bir_lowering=False)

    d = {}
    d["x6T"] = nc.dram_tensor("x6T", [6, NSHARD], bf16, kind="ExternalInput")
    d["idxL"] = nc.dram_tensor("idxL", [128, NG * GS_L // 16], i16, kind="ExternalInput")
    d["idxH"] = nc.dram_tensor("idxH", [128, NG * GS_H // 16], i16, kind="ExternalInput")
    d["ws"] = nc.dram_tensor("ws", [WIN, 128, NCH * 292], bf16, kind="ExternalInput")
    d["sgw"] = nc.dram_tensor("sgw", [WIN, 128, 32], bf16, kind="ExternalInput")
    wshapes = dict(
        kvslab0=[6, KVROW], qcslab0=[6, QCW],
        kvslab1=[161, KVROW], qcslab1=[161, QCW],
        wep0=[20, HID], wep1=[20, HID], prep0=[128, HID], prep1=[128, HID],
        ident=[128, 128], wg1h1=[161, HID], wg1h2=[161, HID],
        wg2rep=[128, HID], bg2rep=[128, 1], wh1=[321, 320], wh2=[321, 6],
    )
    for k, shp in wshapes.items():
        d[k] = nc.dram_tensor(k, shp, bf16, kind="ExternalInput")
    out_d = nc.dram_tensor("out", [32, 6], f32, kind="ExternalOutput")
    dbg_d = nc.dram_tensor("dbg", [128, KVROW], f32, kind="ExternalOutput")

    kv_own = [nc.dram_tensor(f"kv_own{l}", [NSHARD, KVROW], bf16) for l in range(2)]
    kv_full = [nc.dram_tensor(f"kv_full{l}", [NPAD, KVROW], bf16, addr_space="Shared")
               for l in range(2)]
    hT = [None, nc.dram_tensor("hT1", [HID, NSHARD], bf16),
          nc.dram_tensor("hT2", [HID, NSHARD], bf16)]
    h_nm = [None, nc.dram_tensor("h_nm1", [NSHARD, HID], bf16),
            nc.dram_tensor("h_nm2", [NSHARD, HID], bf16)]
    pool_in = nc.dram_tensor("pool_in", [32, 321], f32)
    pool_out = nc.dram_tensor("pool_out", [32, 321], f32, addr_space="Shared")
    rg = [list(range(NCORES))]

    with tile.TileContext(nc) as tc:
        with (
            tc.tile_pool(name="const", bufs=1) as cst,
            tc.tile_pool(name="sb", bufs=2) as sb,
            tc.tile_pool(name="gath", bufs=4) as gath,
            tc.tile_pool(name="ps", bufs=2, space="PSUM") as ps,
        ):
            nc.gpsimd.load_library(mlp)
            regGS_L = nc.gpsimd.to_reg(GS_L)
            regGS_H = nc.gpsimd.to_reg(GS_H)

            C = {}
            def _load_const(key, part, cols, row0=0, dt=bf16):
                t = cst.tile([part, cols], dt, name=f"c_{key}_{row0}")
                nc.sync.dma_start(out=t[:], in_=d[key][row0:row0 + part, :])
                return t
            C["kvslab0"] = _load_const("kvslab0", 6, KVROW)
            C["qcslab0"] = _load_const("qcslab0", 6, QCW)
            C["kvslab1a"] = _load_const("kvslab1", 128, KVROW)
            C["kvslab1b"] = _load_const("kvslab1", 32, KVROW, 128)
            C["kvslab1c"] = _load_const("kvslab1", 1, KVROW, 160)
            C["qcslab1a"] = _load_const("qcslab1", 128, QCW)
            C["qcslab1b"] = _load_const("qcslab1", 32, QCW, 128)
            C["qcslab1c"] = _load_const("qcslab1", 1, QCW, 160)
            for l in range(2):
                C[f"wep{l}"] = _load_const(f"wep{l}", 20, HID)
                C[f"prep{l}"] = _load_const(f"prep{l}", 128, HID)
            C["ident"] = _load_const("ident", 128, 128)
            for key in ("wg1h1", "wg1h2"):
                C[key + "a"] = _load_const(key, 128, HID)
                C[key + "b"] = _load_const(key, 32, HID, 128)
            C["wg1bias"] = _load_const("wg1h1", 1, HID, 160)
            C["wg2rep"] = _load_const("wg2rep", 128, HID)
            C["bg2rep"] = _load_const("bg2rep", 128, 1)
            C["wh1a"] = _load_const("wh1", 128, 320)
            C["wh1b"] = _load_const("wh1", 128, 320, 128)
            C["wh1c"] = _load_const("wh1", 64, 320, 256)
            C["wh1d"] = _load_const("wh1", 1, 320, 320)
            C["wh2a"] = _load_const("wh2", 128, 6)
            C["wh2b"] = _load_const("wh2", 128, 6, 128)
            C["wh2c"] = _load_const("wh2", 64, 6, 256)
            C["wh2d"] = _load_const("wh2", 1, 6, 320)

            idxLt = cst.tile([128, NG * GS_L // 16], i16, name="idxLt")
            nc.sync.dma_start(out=idxLt[:], in_=d["idxL"][:])
            idxHt = cst.tile([128, NG * GS_H // 16], i16, name="idxHt")
            nc.sync.dma_start(out=idxHt[:], in_=d["idxH"][:])

            ones1 = cst.tile([1, 128], bf16, name="ones1")
            nc.gpsimd.memset(ones1[:], 1.0)
            eps4 = cst.tile([128, 4], f32, name="eps4")
            nc.gpsimd.memset(eps4[:], 1e-30)
            onep = cst.tile([128, 1], bf16, name="onep")
            nc.gpsimd.memset(onep[:], 1.0)
            eps32 = cst.tile([32, 1], f32, name="eps32")
            nc.gpsimd.memset(eps32[:], 1e-30)

            for layer in range(2):
                # ---- kv GEMM own nodes -> kv_own (layer-1 kv fused in edge0) ----
                if layer == 0:
                    with nc.named_scope("kv0"):
                        for t in range(WIN):
                            csl = slice(t * 128, (t + 1) * 128)
                            pkv = ps.tile([128, KVROW], f32, tag="kve", bufs=2)
                            xts = sb.tile([6, 128], bf16, tag="xts", bufs=3)
                            nc.sync.dma_start(out=xts[:], in_=d["x6T"][:, csl])
                            nc.tensor.matmul(pkv[:], xts[:], C["kvslab0"][:],
                                             start=True, stop=True)
                            kvsb = sb.tile([128, KVROW], bf16, tag="kvsb")
                            nc.scalar.activation(out=kvsb[:], in_=pkv[:], func=AF.Copy)
                            nc.sync.dma_start(out=kv_own[0][csl, :], in_=kvsb[:])
                with nc.named_scope(f"ag{layer}"):
                    for qi in range(4):
                        nc.gpsimd.collective_compute(
                            "AllGather", mybir.AluOpType.bypass, replica_groups=rg,
                            ins=[kv_own[layer][qi * 1568:(qi + 1) * 1568, :]],
                            outs=[kv_full[layer][qi * 12544:(qi + 1) * 12544, :]])

                # ---- edge phase ----
                with nc.named_scope(f"edge{layer}"):
                    cur = {"L": -1, "H": -1}
                    cur_tile = {"L": None, "H": None}

                    def _gather(region, gt):
                        if cur[region] == gt:
                            return cur_tile[region]
                        idxt, base, gsz, reg = (
                            (idxLt, 0, GS_L, regGS_L) if region == "L"
                            else (idxHt, SPLIT, GS_H, regGS_H)
                        )
                        gtile = gath.tile([128, gsz // 128, KVROW], bf16, tag="g" + region)
                        nc.gpsimd.dma_gather(
                            gtile[:],
                            kv_full[layer][base:base + SPLIT, :],
                            idxt[:, gt * (gsz // 16):(gt + 1) * (gsz // 16)],
                            num_idxs=gsz, num_idxs_reg=reg, elem_size=KVROW)
                        cur[region] = gt
                        cur_tile[region] = gtile
                        return gtile

                    for w in range(WIN):
                        wsl = slice(w * 128, (w + 1) * 128)
                        # window node GEMM -> q|C|r|-rQ
                        psq = ps.tile([128, QCW], f32, tag="pq", bufs=2)
                        if layer == 0:
                            xts = sb.tile([6, 128], bf16, tag="xts", bufs=3)
                            nc.sync.dma_start(out=xts[:], in_=d["x6T"][:, wsl])
                            nc.tensor.matmul(psq[:], xts[:], C["qcslab0"][:],
                                             start=True, stop=True)
                        else:
                            hta = sb.tile([128, 128], bf16, tag="hta", bufs=3)
                            nc.sync.dma_start(out=hta[:], in_=hT[1][0:128, wsl])
                            htb = sb.tile([32, 128], bf16, tag="htb", bufs=3)
                            nc.sync.dma_start(out=htb[:], in_=hT[1][128:160, wsl])
                            nc.tensor.matmul(psq[:], hta[:], C["qcslab1a"][:], start=True, stop=False)
                            nc.tensor.matmul(psq[:], htb[:], C["qcslab1b"][:], start=False, stop=False)
                            nc.tensor.matmul(psq[:], ones1[:, :128], C["qcslab1c"][:], start=False, stop=True)
                        qc = sb.tile([128, QCW], bf16, tag="qc", bufs=2)
                        nc.scalar.activation(out=qc[:], in_=psq[:], func=AF.Copy)

                        wst = sb.tile([128, NCH * 292], bf16, tag="wst", bufs=3)
                        nc.sync.dma_start(out=wst[:], in_=d["ws"][w])
                        stqt = wst[:, 0:NCH * 128]
                        stst = wst[:, NCH * 128:NCH * 256]
                        eact = wst[:, NCH * 256:NCH * 292].rearrange(
                            "p (j c) -> p j c", c=36)

                        gl = _gather("L", w)
                        gh = _gather("H", w)
                        halfL = 0
                        halfH = 0

                        # per-chunk qC one-hot gather matmuls (2 per PSUM bank)
                        qcg = sb.tile([128, NCH, QCROW], bf16, tag="qcg", bufs=2)
                        for pj in range((NCH + 1) // 2):
                            jn = min(2, NCH - pj * 2)
                            pq = ps.tile([128, 2, QCROW], f32, tag="pq", bufs=2)
                            for s in range(jn):
                                j = pj * 2 + s
                                nc.tensor.matmul(pq[:, s, :],
                                                 stqt[:, j * 128:(j + 1) * 128],
                                                 qc[:, 0:QCROW],
                                                 start=True, stop=True,
                                                 skip_group_check=True)
                            nc.scalar.activation(out=qcg[:, pj * 2:pj * 2 + jn, :],
                                                 in_=pq[:, 0:jn, :], func=AF.Copy)

                        # batched DVE per L/H group -- all plain 3D inner-contiguous
                        stage = sb.tile([128, NCH, 192], bf16, tag="stage", bufs=2)
                        al1 = sb.tile([128, NCH * 4], f32, tag="al1", bufs=2)
                        al2 = sb.tile([128, NCH * 4], f32, tag="al2", bufs=2)
                        al = sb.tile([128, NCH * 4], f32, tag="al", bufs=2)
                        wt = sb.tile([128, NCH, 192], bf16, tag="wt", bufs=2)
                        exg = sb.tile([128, NCH, 192], bf16, tag="exg", bufs=2)
                        for (g0, cnt, gt, half) in ((0, C_L, gl, halfL),
                                                    (C_L, C_H, gh, halfH)):
                            kvg = gt[:, half:half + cnt, :]
                            qs = qcg[:, g0:g0 + cnt, :]
                            # q*k -> stage[.., 0:160]
                            nc.vector.tensor_tensor(
                                out=stage[:, g0:g0 + cnt, 0:160],
                                in0=qs[:, :, 0:160],
                                in1=kvg[:, :, 0:160],
                                op=mybir.AluOpType.mult)
                            # ea*C -> stage[.., 160:176]
                            nc.vector.tensor_tensor(
                                out=stage[:, g0:g0 + cnt, 160:176],
                                in0=qs[:, :, 160:176],
                                in1=eact[:, g0:g0 + cnt, 0:16],
                                op=mybir.AluOpType.mult)
                        # alpha = sum_d q*k + sum_c ea*C (whole window)
                        nc.vector.tensor_reduce(
                            out=al1[:], in_=stage[:, :, 0:160]
                                .rearrange("p j (h dd) -> p j h dd", h=4),
                            axis=mybir.AxisListType.X, op=mybir.AluOpType.add)
                        nc.vector.tensor_reduce(
                            out=al2[:], in_=stage[:, :, 160:176]
                                .rearrange("p j (h c) -> p j h c", h=4),
                            axis=mybir.AxisListType.X, op=mybir.AluOpType.add)
                        nc.vector.tensor_add(al[:], al1[:], al2[:])
                        # exp-expand on ACT (broadcast input APs)
                        nc.scalar.activation(
                            out=exg[:, :, 0:160].rearrange("p j (h dd) -> p j h dd", h=4),
                            in_=al[:].rearrange("p (j h o) -> p j h o", h=4, o=1)
                                 .to_broadcast([128, NCH, 4, 40]),
                            func=AF.Exp, scale=INVSQD)
                        nc.scalar.activation(
                            out=exg[:, :, 160:180].rearrange("p j (c h) -> p j c h", c=5),
                            in_=al[:].rearrange("p (j o h) -> p j o h", o=1, h=4)
                                 .to_broadcast([128, NCH, 5, 4]),
                            func=AF.Exp, scale=INVSQD)
                        for (g0, cnt, gt, half) in ((0, C_L, gl, halfL),
                                                    (C_L, C_H, gh, halfH)):
                            kvg = gt[:, half:half + cnt, :]
                            # wt v-block = v_g * ex
                            nc.vector.tensor_tensor(
                                out=wt[:, g0:g0 + cnt, 0:160],
                                in0=kvg[:, :, 160:320],
                                in1=exg[:, g0:g0 + cnt, 0:160],
                                op=mybir.AluOpType.mult)
                            # wt S-block (c,h)-major incl ones col = ea|1 * ex
                            nc.vector.tensor_tensor(
                                out=wt[:, g0:g0 + cnt, 160:180],
                                in0=eact[:, g0:g0 + cnt, 16:36],
                                in1=exg[:, g0:g0 + cnt, 160:180],
                                op=mybir.AluOpType.mult)

                        # scatter: acc[nodes, (h,48)] += st^T @ wt
                        pacc = ps.tile([128, 192], f32, tag="acc", bufs=2)
                        for j in range(NCH):
                            nc.tensor.matmul(pacc[:],
                                             stst[:, j * 128:(j + 1) * 128],
                                             wt[:, j, :],
                                             start=(j == 0), stop=(j == NCH - 1),
                                             skip_group_check=True)

                        # ---- window post ----
                        accsb = sb.tile([128, 192], bf16, tag="accsb")
                        nc.scalar.activation(out=accsb[:], in_=pacc[:], func=AF.Copy)
                        # S correction: transpose accS [128, (c,h)] -> [20,128]
                        pst = ps.tile([20, 128], bf16, tag="tp", bufs=1)
                        nc.tensor.transpose(pst[:], accsb[:, 160:180], C["ident"][:])
                        tS = sb.tile([20, 128], bf16, tag="tS")
                        nc.scalar.activation(out=tS[:], in_=pst[:], func=AF.Copy)
                        pcorr = ps.tile([128, HID], f32, tag="tp", bufs=1)
                        nc.tensor.matmul(pcorr[:], tS[:], C[f"wep{layer}"][:],
                                         start=True, stop=True)
                        # outn = (accv + corr) * 1/denom
                        outn0 = sb.tile([128, HID], bf16, tag="outn0")
                        nc.vector.tensor_tensor(
                            out=outn0[:], in0=accsb[:, 0:160], in1=pcorr[:],
                            op=mybir.AluOpType.add)
                        dmax = sb.tile([128, 4], f32, tag="dmax")
                        nc.vector.tensor_tensor(out=dmax[:], in0=accsb[:, 176:180],
                                                in1=eps4[:], op=mybir.AluOpType.max)
                        denr = sb.tile([128, 4], f32, tag="denr")
                        nc.vector.reciprocal(out=denr[:], in_=dmax[:])
                        outn = sb.tile([128, HID], bf16, tag="outn")
                        nc.vector.tensor_tensor(
                            out=outn[:].rearrange("p (h dd) -> p h dd", h=4),
                            in0=outn0[:].rearrange("p (h dd) -> p h dd", h=4),
                            in1=denr[:].rearrange("p (h o) -> p h o", o=1)
                                .to_broadcast([128, 4, 40]),
                            op=mybir.AluOpType.mult)
                        # beta gate
                        scr = sb.tile([128, HID], bf16, tag="scr")
                        nc.vector.tensor_tensor(out=scr[:], in0=outn[:],
                                                in1=C[f"prep{layer}"][:],
                                                op=mybir.AluOpType.mult)
                        outP = sb.tile([128, 1], f32, tag="outP")
                        nc.vector.tensor_reduce(
                            out=outP[:], in_=scr[:].rearrange("p (a b) -> p a b", a=1),
                            axis=mybir.AxisListType.XY, op=mybir.AluOpType.add)
                        exb = sb.tile([128, 1], bf16, tag="exb")
                        nc.scalar.activation(out=exb[:], in_=outP[:], func=AF.Exp,
                                             scale=-1.0, bias=qc[:, 352:353])
                        betad = sb.tile([128, 1], bf16, tag="betad")
                        nc.vector.tensor_tensor(out=betad[:], in0=exb[:], in1=onep[:],
                                                op=mybir.AluOpType.add)
                        beta = sb.tile([128, 1], bf16, tag="beta")
                        with nc.allow_low_precision(reason="beta gate bf16 ok"):
                            nc.vector.reciprocal(out=beta[:], in_=betad[:])
                        dvec = sb.tile([128, HID], bf16, tag="dvec")
                        nc.vector.tensor_sub(dvec[:], qc[:, QCROW:QCROW + 160], outn[:])
                        hp = sb.tile([128, HID], bf16, tag="hp")
                        nc.vector.scalar_tensor_tensor(
                            out=hp[:], in0=dvec[:], scalar=beta[:, 0:1], in1=outn[:],
                            op0=mybir.AluOpType.mult, op1=mybir.AluOpType.add)
                        nc.sync.dma_start(out=h_nm[layer + 1][wsl, :], in_=hp[:])
                        ptr1 = ps.tile([128, 128], bf16, tag="tp", bufs=1)
                        nc.tensor.transpose(ptr1[:], hp[:, 0:128], C["ident"][:])
                        t1 = sb.tile([128, 128], bf16, tag="t1")
                        nc.scalar.activation(out=t1[:], in_=ptr1[:], func=AF.Copy)
                        nc.sync.dma_start(out=hT[layer + 1][0:128, wsl], in_=t1[:])
                        ptr2 = ps.tile([32, 128], bf16, tag="tp", bufs=1)
                        nc.tensor.transpose(ptr2[:], hp[:, 128:160], C["ident"][:])
                        t2 = sb.tile([32, 128], bf16, tag="t2")
                        nc.scalar.activation(out=t2[:], in_=ptr2[:], func=AF.Copy)
                        nc.sync.dma_start(out=hT[layer + 1][128:160, wsl], in_=t2[:])
                        if layer == 0:
                            # kv1 GEMM for this window from the fresh h1^T tiles
                            pkv = ps.tile([128, KVROW], f32, tag="kve", bufs=2)
                            nc.tensor.matmul(pkv[:], t1[:], C["kvslab1a"][:],
                                             start=True, stop=False)
                            nc.tensor.matmul(pkv[:], t2[:], C["kvslab1b"][:],
                                             start=False, stop=False)
                            nc.tensor.matmul(pkv[:], ones1[:, :128], C["kvslab1c"][:],
                                             start=False, stop=True)
                            kvsb = sb.tile([128, KVROW], bf16, tag="kvsb")
                            nc.scalar.activation(out=kvsb[:], in_=pkv[:], func=AF.Copy)
                            nc.sync.dma_start(out=kv_own[1][wsl, :], in_=kvsb[:])

            # ==== final phase: gate + graph pooling + head MLP ====
            with nc.named_scope("final"):
                pgr = ps.tile([32, 321], f32, tag="pgr", bufs=1)
                for w in range(WIN):
                    wsl = slice(w * 128, (w + 1) * 128)
                    h1w = sb.tile([128, HID], bf16, tag="h1w")
                    nc.sync.dma_start(out=h1w[:], in_=h_nm[1][wsl, :])
                    h2w = sb.tile([128, HID], bf16, tag="h2w")
                    nc.sync.dma_start(out=h2w[:], in_=h_nm[2][wsl, :])
                    sgt = sb.tile([128, 32], bf16, tag="sgt", bufs=3)
                    nc.sync.dma_start(out=sgt[:], in_=d["sgw"][w])
                    pg = ps.tile([128, HID], f32, tag="kve", bufs=2)
                    first = True
                    for (src_hT, wkey) in ((hT[1], "wg1h1"), (hT[2], "wg1h2")):
                        g_a = sb.tile([128, 128], bf16, tag="hta", bufs=3)
                        nc.sync.dma_start(out=g_a[:], in_=src_hT[0:128, wsl])
                        g_b = sb.tile([32, 128], bf16, tag="htb", bufs=3)
                        nc.sync.dma_start(out=g_b[:], in_=src_hT[128:160, wsl])
                        nc.tensor.matmul(pg[:], g_a[:], C[wkey + "a"][:], start=first, stop=False)
                        first = False
                        nc.tensor.matmul(pg[:], g_b[:], C[wkey + "b"][:], start=False, stop=False)
                    nc.tensor.matmul(pg[:], ones1[:, :128], C["wg1bias"][:], start=False, stop=True)
                    grelu = sb.tile([128, HID], bf16, tag="grelu")
                    nc.scalar.activation(out=grelu[:], in_=pg[:], func=AF.Relu)
                    scr2 = sb.tile([128, HID], bf16, tag="scr")
                    gatec = sb.tile([128, 1], f32, tag="gatec")
                    nc.vector.tensor_tensor(out=scr2[:], in0=grelu[:],
                                            in1=C["wg2rep"][:], op=mybir.AluOpType.mult)
                    nc.vector.tensor_reduce(
                        out=gatec[:], in_=scr2[:].rearrange("p (a b) -> p a b", a=1),
                        axis=mybir.AxisListType.XY, op=mybir.AluOpType.add)
                    ge = sb.tile([128, 1], f32, tag="ge")
                    nc.scalar.activation(out=ge[:], in_=gatec[:], func=AF.Exp,
                                         bias=C["bg2rep"][:, 0:1])
                    wg = sb.tile([128, 321], bf16, tag="wg")
                    nc.vector.tensor_tensor(
                        out=wg[:, 0:HID], in0=h1w[:],
                        in1=ge[:].to_broadcast([128, HID]), op=mybir.AluOpType.mult)
                    nc.vector.tensor_tensor(
                        out=wg[:, HID:2 * HID], in0=h2w[:],
                        in1=ge[:].to_broadcast([128, HID]), op=mybir.AluOpType.mult)
                    nc.vector.tensor_copy(out=wg[:, 320:321], in_=ge[:])
                    nc.tensor.matmul(pgr[:], sgt[:], wg[:], start=(w == 0),
                                     stop=(w == WIN - 1), skip_group_check=True)
                pg_sb = sb.tile([32, 321], f32, tag="pg_sb")
                nc.vector.tensor_copy(out=pg_sb[:], in_=pgr[:])
                nc.sync.dma_start(out=pool_in[:], in_=pg_sb[:])
                nc.gpsimd.collective_compute(
                    "AllReduce", mybir.AluOpType.add, replica_groups=rg,
                    ins=[pool_in[:]], outs=[pool_out[:]])
                psb = sb.tile([32, 321], f32, tag="psb")
                nc.sync.dma_start(out=psb[:], in_=pool_out[:])
                gden = sb.tile([32, 1], f32, tag="gden")
                nc.vector.tensor_tensor(out=gden[:], in0=psb[:, 320:321],
                                        in1=eps32[:], op=mybir.AluOpType.max)
                gdr = sb.tile([32, 1], f32, tag="gdr")
                nc.vector.reciprocal(out=gdr[:], in_=gden[:])
                pl = sb.tile([32, 320], bf16, tag="pl")
                nc.vector.tensor_tensor(
                    out=pl[:], in0=psb[:, 0:320],
                    in1=gdr[:].to_broadcast([32, 320]), op=mybir.AluOpType.mult)

                def _headmm(vin, wa, wb, wc, wd, nout, tagp):
                    pouts = ps.tile([32, nout], f32, tag=tagp, bufs=2)
                    for si, (c0, m) in enumerate(((0, 128), (128, 128), (256, 64))):
                        ptt = ps.tile([m, 32], bf16, tag="tp", bufs=1)
                        nc.tensor.transpose(ptt[:], vin[:, c0:c0 + m], C["ident"][0:32, 0:32])
                        tsb = sb.tile([m, 32], bf16, tag="tsb")
                        nc.vector.tensor_copy(out=tsb[:], in_=ptt[:])
                        nc.tensor.matmul(pouts[:], tsb[:], (wa, wb, wc)[si][:m, :],
                                         start=(si == 0), stop=False, skip_group_check=True)
                    nc.tensor.matmul(pouts[:], ones1[:, :32], wd[:],
                                     start=False, stop=True, skip_group_check=True)
                    return pouts

                ph1 = _headmm(pl, C["wh1a"], C["wh1b"], C["wh1c"], C["wh1d"], 320, "pq")
                vrel = sb.tile([32, 320], bf16, tag="vrel")
                nc.scalar.activation(out=vrel[:], in_=ph1[:], func=AF.Relu)
                ph2 = _headmm(vrel, C["wh2a"], C["wh2b"], C["wh2c"], C["wh2d"], 6, "kve")
                osb = sb.tile([32, 6], f32, tag="osb")
                nc.vector.tensor_copy(out=osb[:], in_=ph2[:])
                nc.sync.dma_start(out=out_d[:], in_=osb[:])
                dbgt = sb.tile([128, KVROW], f32, tag="dbgt")
                nc.gpsimd.memset(dbgt[:], 0.0)
                nc.sync.dma_start(out=dbg_d[:], in_=dbgt[:])

    nc.compile()
    return nc


_CACHE = {}
_LAST_RES = None


def kernel(**inputs):
    inputs = {k: np.asarray(v) for k, v in inputs.items()}
    per_core, C_L, C_H = _preprocess(
        inputs["x"], inputs["edge_index"], inputs["edge_attr"], inputs["batch"])
    w = _weights(inputs)
    key = (C_L, C_H)
    if key not in _CACHE:
        _CACHE[key] = _build(C_L, C_H)
    nc = _CACHE[key]
    in_maps = []
    for r in range(NCORES):
        m = dict(w)
        m.update(per_core[r])
        in_maps.append(m)
    import os
    trace = bool(os.environ.get("KERNEL_TRACE"))
    if trace:
        try:
            import axon_prof
            axon_prof.install()
        except Exception:
            trace = False
    res = run_bass_kernel_spmd(nc, in_maps, core_ids=list(range(NCORES)), trace=trace)
    if trace and res.exec_time_ns is not None:
        print(f"HW exec time: {res.exec_time_ns} ns")
        if res.per_core_scope_times:
            for scope, cores in sorted(res.per_core_scope_times.items()):
                print(f"  scope {scope}: {cores}")
    global _LAST_RES
    _LAST_RES = res
    out = res.results[0]["out"]
    return out.reshape(G, 2, 3).astype(np.float32)
